# revision 1
# baseline (speedup 1.0000x reference)
"""CenterNet loss (GT assignment + focal/giou losses) on 8 Trainium2 cores.

Sharding: core c handles image b = c//2 and half h = c%2 of EVERY FPN level
(so all 8 cores run an identical SPMD tile schedule). Each core produces
partial sums (giou_sum, reg_cnt, pos_sum, neg_sum, npos); a DRAM AllReduce
combines them and every core computes the final 3-vector.
"""

import numpy as np

import concourse.bass as bass
import concourse.bacc as bacc
import concourse.tile as tile
from concourse import ap_utils, mybir
from concourse.bass_utils import run_bass_kernel_spmd


def _pool_on(eng, nc, out, in_, func):
    """Emit InstPool (innermost-dim reduction) on the given engine.

    Pads the input AP to 5-D (hardware requirement) via unsqueeze."""
    while len(in_.shape) < 5:
        in_ = in_.unsqueeze(1)
    return eng.add_instruction(mybir.InstPool(
        name=f"I-{nc.next_id()}", func=func,
        ins=[eng.lower_ap(in_, opt=False)], outs=[eng.lower_ap(out)]))

F32 = mybir.dt.float32
I32 = mybir.dt.int32
AF = mybir.ActivationFunctionType
OP = mybir.AluOpType
AX = mybir.AxisListType

# ---------------- problem constants (hardcoded from the nn.Module) ---------
B, NBOX = 4, 64
STRIDES = (8, 16, 32, 64, 128)
LEVEL_HW = ((128, 128), (64, 64), (32, 32), (16, 16), (8, 8))
SIZES = ((0.0, 80.0), (64.0, 160.0), (128.0, 320.0), (256.0, 640.0), (512.0, 1e7))
LOC = [h * w for h, w in LEVEL_HW]          # [16384, 4096, 1024, 256, 64]
M_IMG = sum(LOC)                            # 21824
M_TOT = B * M_IMG                           # 87296
BASE = [0, 65536, 81920, 86016, 87040]      # global level bases (level-major)
HALF = [m // 2 for m in LOC]                # per-core per-level loc counts
NT = 86                                     # 128-loc tiles per core
NV = sum(HALF)                              # 10912 valid locs per core
NPAD = NT * 128                             # 11008
INF = 1e8
MIN_RADIUS2 = 16.0
DELTA = (1 - 0.8) / (1 + 0.8)
K_R2 = float(np.float32(DELTA ** 2 * 2))    # radius2 = max(K_R2*area, 16)
SIG_LO = float(np.float32(1e-4))
SIG_HI = float(np.float32(1.0 - 1e-4))
EPS_AC = float(np.float32(1e-7))
IGNORE_HIGH_FP = 0.85
MAGIC = 8388608.0  # 2^23: u+MAGIC-MAGIC rounds u to nearest int (u < 2^22)
# supergroups: (tile0, n_tiles, level); all tiles in a group share a level
SG = [(i * 8, 8, 0) for i in range(8)] + [
    (64, 8, 1), (72, 8, 1), (80, 4, 2), (84, 1, 3), (85, 1, 4)]
# tiles per level: L0 t0-63, L1 64-79, L2 80-83, L3 84, L4 85 (32 valid rows)

N_CORES = 8


def _pack(vec):
    """[NPAD] (loc j = t*128+p) -> [128, NT] with [p, t] layout."""
    return np.ascontiguousarray(vec.reshape(NT, 128).T)


def _grids_per_level():
    gs = []
    for (h, w), s in zip(LEVEL_HW, STRIDES):
        ys, xs = np.meshgrid(np.arange(h) * s, np.arange(w) * s, indexing="ij")
        g = np.stack([xs.reshape(-1), ys.reshape(-1)], 1).astype(np.float32) + s // 2
        gs.append(g)
    return gs


def _half_concat(per_level_fn, h):
    """Concat per-level arrays for half h, pad to NPAD."""
    parts = [per_level_fn(l, h) for l in range(5)]
    cat = np.concatenate(parts, 0)
    pad_shape = (NPAD - NV,) + cat.shape[1:]
    return np.concatenate([cat, np.zeros(pad_shape, cat.dtype)], 0)


_GRIDS = _grids_per_level()


def _build_locstat(h):
    """[128, 8, NT]: planes gx, gy, gx, gy, -gx, -gy, valid, inv_s."""
    g = _half_concat(lambda l, hh: _GRIDS[l][hh * HALF[l]:(hh + 1) * HALF[l]], h)
    gx, gy = g[:, 0], g[:, 1]
    valid = np.zeros(NPAD, np.float32)
    valid[:NV] = 1.0
    inv_s = _half_concat(
        lambda l, hh: np.full(HALF[l], 1.0 / STRIDES[l], np.float32), h)
    inv_s[NV:] = 1.0
    planes = [gx, gy, gx, gy, -gx, -gy, valid, inv_s]
    out = np.stack([_pack(p.astype(np.float32)) for p in planes], 1)
    return np.ascontiguousarray(out)  # [128, 8, NT]


_LOCSTAT = [_build_locstat(0), _build_locstat(1)]

# iota constants [128, 2, 64] bf16: plane0 = 0..63, plane1 = 64 - iota
import ml_dtypes
_MISC = np.ascontiguousarray(np.stack([
    np.broadcast_to(np.arange(64, dtype=np.float32), (128, 64)),
    np.broadcast_to(64.0 - np.arange(64, dtype=np.float32), (128, 64)),
], 1)).astype(ml_dtypes.bfloat16)


def _shard_idx(b, h):
    """Global level-major indices of core (b, h)'s NV locations."""
    parts = [BASE[l] + b * LOC[l] + h * HALF[l] + np.arange(HALF[l])
             for l in range(5)]
    return np.concatenate(parts, 0)


_SHARD_IDX = {(b, h): _shard_idx(b, h) for b in range(B) for h in range(2)}


# ------------------------------ device program -----------------------------

def build_nc(with_cc=True, dbg=False):
    nc = bacc.Bacc(trn_type="TRN2", num_devices=N_CORES)
    locst = nc.dram_tensor("locst", [128, 8, NT], F32, kind="ExternalInput")
    dyn = nc.dram_tensor("dyn", [128, 5, NT], F32, kind="ExternalInput")
    boxesT = nc.dram_tensor("boxesT", [4, NBOX], F32, kind="ExternalInput")
    boxesP = nc.dram_tensor("boxesP", [2 * NBOX, 4], F32, kind="ExternalInput")
    agnfull = nc.dram_tensor("agnfull", [M_TOT, 1], F32, kind="ExternalInput")
    corec = nc.dram_tensor("corec", [NBOX, 8], F32, kind="ExternalInput")
    miscc = nc.dram_tensor("miscc", [128, 2, 64], mybir.dt.bfloat16,
                           kind="ExternalInput")
    out = nc.dram_tensor("out", [3], F32, kind="ExternalOutput")
    if dbg:
        pdbg = nc.dram_tensor("pdbg", [1, 8], F32, kind="ExternalOutput")
        minddbg = nc.dram_tensor("minddbg", [128, NT], F32, kind="ExternalOutput")
        minwdbg = nc.dram_tensor("minwdbg", [128, NT], F32, kind="ExternalOutput")
        xtdbg = nc.dram_tensor("xtdbg", [128, 4, NT], F32, kind="ExternalOutput")
        posdbg = nc.dram_tensor("posdbg", [NBOX, 5], F32, kind="ExternalOutput")
        gvdbg = nc.dram_tensor("gvdbg", [NBOX, 5], F32, kind="ExternalOutput")
    if with_cc:
        pdram = nc.dram_tensor("pdram", [1, 8], F32)
        ccout = nc.dram_tensor("ccout", [1, 8], F32, addr_space="Shared")

    vec, act, gps, sync = nc.vector, nc.scalar, nc.gpsimd, nc.sync

    with tile.TileContext(nc) as tc:
        with tc.tile_pool(name="const", bufs=1) as cp, \
             tc.tile_pool(name="work", bufs=3) as wp, \
             tc.tile_pool(name="ppool", bufs=2, space="PSUM") as pp:

            # ---------------- loads ----------------
            SL = cp.tile([128, 8, NT], F32)
            sync.dma_start(out=SL[:], in_=locst[:])
            DY = cp.tile([128, 5, NT], F32)
            sync.dma_start(out=DY[:], in_=dyn[:])
            BBh = boxesT[:]
            BB = cp.tile([128, 4, NBOX], F32)
            bt_bc = bass.AP(tensor=BBh.tensor, offset=BBh.offset,
                            ap=[[0, 128], [NBOX, 4], [1, NBOX]])
            sync.dma_start(out=BB[:], in_=bt_bc)
            BP = cp.tile([2 * NBOX, 4], F32)
            sync.dma_start(out=BP[:], in_=boxesP[:])
            CO = cp.tile([NBOX, 8], F32)
            sync.dma_start(out=CO[:], in_=corec[:])
            MI = cp.tile([128, 2, 64], mybir.dt.bfloat16)
            sync.dma_start(out=MI[:], in_=miscc[:])
            IOTA = MI[:, 0, :]
            IOTAN = MI[:, 1, :]

            # ---------------- per-box precompute ([128, 64] broadcast) -----
            from concourse.masks import make_identity
            IDT = cp.tile([128, 128], mybir.dt.bfloat16)
            make_identity(nc, IDT[:])
            x0, y0 = BB[:, 0, :], BB[:, 1, :]
            x1, y1 = BB[:, 2, :], BB[:, 3, :]
            CXY = cp.tile([128, 2, NBOX], F32)
            CX, CY = CXY[:, 0, :], CXY[:, 1, :]
            vec.tensor_tensor(out=CXY[:], in0=BB[:, 0:2, :], in1=BB[:, 2:4, :],
                              op=OP.add)
            vec.tensor_scalar(out=CXY[:], in0=CXY[:], scalar1=0.5, scalar2=None,
                              op0=OP.mult)
            # W2C = (w/2, h/2) per box
            W2C = cp.tile([128, 2, NBOX], F32)
            vec.tensor_tensor(out=W2C[:], in0=BB[:, 2:4, :], in1=BB[:, 0:2, :],
                              op=OP.subtract)
            vec.tensor_scalar(out=W2C[:], in0=W2C[:], scalar1=0.5, scalar2=None,
                              op0=OP.mult)
            S1 = cp.tile([128, NBOX], F32)
            S2 = cp.tile([128, NBOX], F32)
            S3 = cp.tile([128, NBOX], F32)
            # radius2 = max(K_R2 * area, 16);  IR2N = -1/radius2
            R2 = cp.tile([128, NBOX], F32)
            vec.tensor_tensor(out=S1[:], in0=x1, in1=x0, op=OP.subtract)  # w
            vec.tensor_tensor(out=S2[:], in0=y1, in1=y0, op=OP.subtract)  # h
            vec.tensor_tensor(out=R2[:], in0=S1[:], in1=S2[:], op=OP.mult)
            vec.tensor_scalar(out=R2[:], in0=R2[:], scalar1=K_R2,
                              scalar2=MIN_RADIUS2, op0=OP.mult, op1=OP.max)
            IR2N = cp.tile([128, NBOX], F32)
            vec.reciprocal(out=IR2N[:], in_=R2[:])
            vec.tensor_scalar(out=IR2N[:], in0=IR2N[:], scalar1=-1.0,
                              scalar2=None, op0=OP.mult)
            # crit = sqrt(w^2 + h^2) / 2  (per box)
            CRIT = cp.tile([128, NBOX], F32)
            vec.tensor_tensor(out=S1[:], in0=S1[:], in1=S1[:], op=OP.mult)
            vec.tensor_tensor(out=S2[:], in0=S2[:], in1=S2[:], op=OP.mult)
            vec.tensor_tensor(out=S1[:], in0=S1[:], in1=S2[:], op=OP.add)
            act.activation(out=CRIT[:], in_=S1[:], func=AF.Sqrt, scale=0.25)
            # PBS [128, 4] = (x0, y0, -x1, -y1) boxes-on-partitions
            PBS = cp.tile([2 * NBOX, 4], F32)
            vec.tensor_copy(out=PBS[:, 0:2], in_=BP[:, 0:2])
            vec.tensor_scalar(out=PBS[:, 2:4], in0=BP[:, 2:4], scalar1=-1.0,
                              scalar2=None, op0=OP.mult)
            # exact bf16 triple-split of PBS: hi+mid+lo == PBS bitwise in f32
            BF16 = mybir.dt.bfloat16
            PBS3 = cp.tile([NBOX, 3, 4], BF16)
            PR1 = cp.tile([NBOX, 4], F32)
            PR2 = cp.tile([NBOX, 4], F32)
            vec.tensor_copy(out=PBS3[:, 0, :], in_=PBS[0:NBOX, :])
            vec.tensor_copy(out=PR1[:], in_=PBS3[:, 0, :])  # hi back to f32
            vec.tensor_tensor(out=PR1[:], in0=PBS[0:NBOX, :], in1=PR1[:],
                              op=OP.subtract)
            vec.tensor_copy(out=PBS3[:, 1, :], in_=PR1[:])
            vec.tensor_copy(out=PR2[:], in_=PBS3[:, 1, :])
            vec.tensor_tensor(out=PR2[:], in0=PR1[:], in1=PR2[:], op=OP.subtract)
            vec.tensor_copy(out=PBS3[:, 2, :], in_=PR2[:])
            # per-level: BCNN[l] = -INF*carednot; CB4[l] = (cx, cy, cdisx, cdisy)
            BCNN, CB4 = [], []
            for l in range(5):
                s = float(STRIDES[l])
                lo, hi = SIZES[l]
                cn = cp.tile([128, NBOX], F32, tag=f"bcnn{l}", name=f"bcnn{l}")
                vec.tensor_scalar(out=S1[:], in0=CRIT[:], scalar1=float(lo),
                                  scalar2=-INF, op0=OP.is_lt, op1=OP.mult)
                vec.tensor_scalar(out=cn[:], in0=CRIT[:], scalar1=float(hi),
                                  scalar2=-INF, op0=OP.is_gt, op1=OP.mult)
                vec.tensor_tensor(out=cn[:], in0=cn[:], in1=S1[:], op=OP.min)
                BCNN.append(cn)
                t = cp.tile([128, 4, NBOX], F32, tag=f"cb4{l}", name=f"cb4{l}")
                vec.tensor_copy(out=t[:, 0:2, :], in_=CXY[:])
                for ci, src_ in enumerate((CX, CY)):
                    vec.tensor_scalar(out=S1[:], in0=src_, scalar1=1.0 / s,
                                      scalar2=None, op0=OP.mult)  # u = c/s
                    vec.tensor_scalar(out=S2[:], in0=S1[:], scalar1=MAGIC,
                                      scalar2=None, op0=OP.add)
                    vec.tensor_scalar(out=S2[:], in0=S2[:], scalar1=-MAGIC,
                                      scalar2=None, op0=OP.add)
                    vec.tensor_tensor(out=S3[:], in0=S2[:], in1=S1[:], op=OP.is_gt)
                    vec.tensor_tensor(out=S2[:], in0=S2[:], in1=S3[:],
                                      op=OP.subtract)  # floor(c/s)
                    vec.tensor_scalar(out=t[:, 2 + ci, :], in0=S2[:], scalar1=s,
                                      scalar2=s / 2.0, op0=OP.mult, op1=OP.add)
                CB4.append(t)

            # ---------------- pos part (boxes on partitions, [64, *]) -------
            cx = cp.tile([NBOX, 1], F32, tag="pcx", name="pcx")
            cy = cp.tile([NBOX, 1], F32, tag="pcy", name="pcy")
            vec.tensor_tensor(out=cx[:], in0=BP[0:NBOX, 0:1], in1=BP[0:NBOX, 2:3], op=OP.add)
            vec.tensor_scalar(out=cx[:], in0=cx[:], scalar1=0.5, scalar2=None,
                              op0=OP.mult)
            vec.tensor_tensor(out=cy[:], in0=BP[0:NBOX, 1:2], in1=BP[0:NBOX, 3:4], op=OP.add)
            vec.tensor_scalar(out=cy[:], in0=cy[:], scalar1=0.5, scalar2=None,
                              op0=OP.mult)
            pw = cp.tile([NBOX, 1], F32, tag="ppw", name="ppw")
            ph = cp.tile([NBOX, 1], F32, tag="pph", name="pph")
            vec.tensor_tensor(out=pw[:], in0=BP[0:NBOX, 2:3], in1=BP[0:NBOX, 0:1],
                              op=OP.subtract)
            vec.tensor_tensor(out=ph[:], in0=BP[0:NBOX, 3:4], in1=BP[0:NBOX, 1:2],
                              op=OP.subtract)
            vec.tensor_tensor(out=pw[:], in0=pw[:], in1=pw[:], op=OP.mult)
            vec.tensor_tensor(out=ph[:], in0=ph[:], in1=ph[:], op=OP.mult)
            vec.tensor_tensor(out=pw[:], in0=pw[:], in1=ph[:], op=OP.add)
            pcrit = cp.tile([NBOX, 1], F32, tag="pcrit", name="pcrit")
            act.activation(out=pcrit[:], in_=pw[:], func=AF.Sqrt, scale=0.25)
            POSF = cp.tile([NBOX, 5], F32, tag="posf", name="posf")
            PM = cp.tile([NBOX, 5], F32, tag="pm", name="pm")
            pa_ = cp.tile([NBOX, 1], F32, tag="pa_", name="pa_")
            pb_ = cp.tile([NBOX, 1], F32, tag="pb_", name="pb_")
            pc_ = cp.tile([NBOX, 1], F32, tag="pc_", name="pc_")
            pd_ = cp.tile([NBOX, 1], F32, tag="pd_", name="pd_")
            pe_ = cp.tile([NBOX, 1], F32, tag="pe_", name="pe_")
            for l in range(5):
                s = float(STRIDES[l])
                w_l = float(LEVEL_HW[l][1])
                lo, hi = SIZES[l]
                # ci_x = floor(cx/s) via round-then-correct
                vec.tensor_scalar(out=pd_[:], in0=cx[:], scalar1=1.0 / s,
                                  scalar2=None, op0=OP.mult)
                vec.tensor_scalar(out=pa_[:], in0=pd_[:], scalar1=MAGIC,
                                  scalar2=None, op0=OP.add)
                vec.tensor_scalar(out=pa_[:], in0=pa_[:], scalar1=-MAGIC,
                                  scalar2=None, op0=OP.add)
                vec.tensor_tensor(out=pe_[:], in0=pa_[:], in1=pd_[:], op=OP.is_gt)
                vec.tensor_tensor(out=pa_[:], in0=pa_[:], in1=pe_[:],
                                  op=OP.subtract)  # ci_x
                vec.tensor_scalar(out=pd_[:], in0=cy[:], scalar1=1.0 / s,
                                  scalar2=None, op0=OP.mult)
                vec.tensor_scalar(out=pb_[:], in0=pd_[:], scalar1=MAGIC,
                                  scalar2=None, op0=OP.add)
                vec.tensor_scalar(out=pb_[:], in0=pb_[:], scalar1=-MAGIC,
                                  scalar2=None, op0=OP.add)
                vec.tensor_tensor(out=pe_[:], in0=pb_[:], in1=pd_[:], op=OP.is_gt)
                vec.tensor_tensor(out=pb_[:], in0=pb_[:], in1=pe_[:],
                                  op=OP.subtract)  # ci_y
                vec.tensor_scalar(out=pc_[:], in0=pb_[:], scalar1=w_l,
                                  scalar2=None, op0=OP.mult)
                vec.tensor_tensor(out=pc_[:], in0=pc_[:], in1=pa_[:], op=OP.add)
                vec.tensor_scalar(out=pc_[:], in0=pc_[:], scalar1=CO[:, l:l + 1],
                                  scalar2=None, op0=OP.add)
                vec.tensor_scalar(out=POSF[:, l:l + 1], in0=pc_[:], scalar1=0.0,
                                  scalar2=float(M_TOT - 1), op0=OP.max, op1=OP.min)
                vec.tensor_scalar(out=pa_[:], in0=pcrit[:], scalar1=float(lo),
                                  scalar2=None, op0=OP.is_ge)
                vec.scalar_tensor_tensor(out=PM[:, l:l + 1], in0=pcrit[:],
                                         scalar=float(hi), in1=pa_[:],
                                         op0=OP.is_le, op1=OP.mult)
            POSI = cp.tile([NBOX, 5], I32, tag="posi", name="posi")
            vec.tensor_copy(out=POSI[:], in_=POSF[:])
            GV = cp.tile([NBOX, 5], F32, tag="gv", name="gv")
            for l in range(5):
                gps.indirect_dma_start(
                    out=GV[:, l:l + 1], out_offset=None, in_=agnfull[:],
                    in_offset=bass.IndirectOffsetOnAxis(ap=POSI[:, l:l + 1],
                                                        axis=0))
            PPRED = cp.tile([NBOX, 5], F32, tag="ppred", name="ppred")
            act.activation(out=PPRED[:], in_=GV[:], func=AF.Sigmoid)
            vec.tensor_scalar(out=PPRED[:], in0=PPRED[:], scalar1=SIG_LO,
                              scalar2=SIG_HI, op0=OP.max, op1=OP.min)
            QQ = cp.tile([NBOX, 5], F32, tag="qq", name="qq")
            vec.tensor_scalar(out=QQ[:], in0=PPRED[:], scalar1=-1.0, scalar2=1.0,
                              op0=OP.mult, op1=OP.add)
            vec.tensor_tensor(out=QQ[:], in0=QQ[:], in1=QQ[:], op=OP.mult)
            LGP = cp.tile([NBOX, 5], F32, tag="lgp", name="lgp")
            act.activation(out=LGP[:], in_=PPRED[:], func=AF.Ln)
            vec.tensor_tensor(out=LGP[:], in0=LGP[:], in1=QQ[:], op=OP.mult)
            vec.tensor_tensor(out=LGP[:], in0=LGP[:], in1=PM[:], op=OP.mult)
            # gate odd cores to zero (pos part owned by even core of each image)
            vec.tensor_scalar(out=LGP[:], in0=LGP[:], scalar1=CO[:, 5:6],
                              scalar2=None, op0=OP.mult)
            vec.tensor_scalar(out=PM[:], in0=PM[:], scalar1=CO[:, 5:6],
                              scalar2=None, op0=OP.mult)
            POSS = cp.tile([NBOX, 1], F32, tag="poss", name="poss")
            vec.tensor_reduce(out=POSS[:], in_=LGP[:], axis=AX.X, op=OP.add)
            NPOS = cp.tile([NBOX, 1], F32, tag="npos", name="npos")
            vec.tensor_reduce(out=NPOS[:], in_=PM[:], axis=AX.X, op=OP.add)


            # ---------------- main pair loop --------------------------------
            # negated-min convention: MINWN = -min(wdist2), MINDN = -min(d),
            # NIXP = 4096 - argmin(d).
            MINWN = cp.tile([128, NT], F32)
            MINDN = cp.tile([128, NT], F32)
            NIXP = cp.tile([128, NT], mybir.dt.bfloat16)
            XT = cp.tile([128, 4, NT], F32)  # selected (x0, y0, -x1, -y1)

            for (t0, G, l) in SG:
                s = float(STRIDES[l])
                sl = slice(t0, t0 + G)

                def bb1(t2d):  # [128,64] const -> [128,G,64]
                    return t2d.unsqueeze(1).broadcast_to((128, G, 64))

                # DF = (gx-cx, gy-cy, gx-cdisx, gy-cdisy)
                DF = wp.tile([128, 4, G, 64], F32, tag="df", name="df")
                vec.tensor_tensor(
                    out=DF[:],
                    in0=SL[:, 0:4, sl].unsqueeze(3).broadcast_to((128, 4, G, 64)),
                    in1=CB4[l][:].unsqueeze(2).broadcast_to((128, 4, G, 64)),
                    op=OP.subtract)
                AB4 = wp.tile([128, 4, G, 64], F32, tag="ab4", name="ab4")
                act.activation(out=AB4[:], in_=DF[:], func=AF.Abs)
                SQ2 = wp.tile([128, 2, G, 64], F32, tag="sq2", name="sq2")
                act.square(out=SQ2[:], in_=DF[:, 0:2])
                # min(l,t,r,b) = min over xy of (w/2 - |dx|, h/2 - |dy|)
                MXY = wp.tile([128, 2, G, 64], F32, tag="mxy", name="mxy")
                vec.tensor_tensor(
                    out=MXY[:],
                    in0=W2C[:].unsqueeze(2).broadcast_to((128, 2, G, 64)),
                    in1=AB4[:, 0:2], op=OP.subtract)
                M4 = wp.tile([128, G, 64], F32, tag="m4", name="m4")
                vec.tensor_tensor(out=M4[:], in0=MXY[:, 0], in1=MXY[:, 1],
                                  op=OP.min)
                VIN = wp.tile([128, G, 64], F32, tag="vin", name="vin")
                vec.tensor_scalar(out=VIN[:], in0=M4[:], scalar1=0.0,
                                  scalar2=-INF, op0=OP.is_le, op1=OP.mult)
                D2 = wp.tile([128, G, 64], F32, tag="d2", name="d2")
                vec.tensor_tensor(out=D2[:], in0=SQ2[:, 0], in1=SQ2[:, 1],
                                  op=OP.add)
                M = wp.tile([128, G, 64], F32, tag="m", name="m")
                vec.tensor_tensor(out=M[:], in0=AB4[:, 2], in1=AB4[:, 3],
                                  op=OP.max)
                DZ = wp.tile([128, G, 64], F32, tag="dz", name="dz")
                vec.scalar_tensor_tensor(out=DZ[:], in0=M[:], scalar=0.0,
                                         in1=D2[:], op0=OP.not_equal, op1=OP.mult)
                WDN = wp.tile([128, G, 64], F32, tag="wdn", name="wdn")
                vec.tensor_tensor(out=WDN[:], in0=DZ[:], in1=bb1(IR2N[:]),
                                  op=OP.mult)  # -wdist2
                vec.tensor_reduce(out=MINWN[:, sl], in_=WDN[:], axis=AX.X,
                                  op=OP.max)
                VC3 = wp.tile([128, G, 64], F32, tag="vc3", name="vc3")
                vec.tensor_scalar(out=VC3[:], in0=M[:], scalar1=s, scalar2=-INF,
                                  op0=OP.is_gt, op1=OP.mult)
                vec.tensor_tensor(out=VC3[:], in0=VIN[:], in1=VC3[:], op=OP.min)
                vec.tensor_tensor(out=VC3[:], in0=VC3[:], in1=bb1(BCNN[l][:]),
                                  op=OP.min)
                DN = wp.tile([128, G, 64], F32, tag="dn", name="dn")
                vec.tensor_tensor(out=DN[:], in0=VC3[:], in1=WDN[:], op=OP.min)
                vec.tensor_reduce(out=MINDN[:, sl], in_=DN[:], axis=AX.X,
                                  op=OP.max)
                EQ = wp.tile([128, G, 64], mybir.dt.bfloat16, tag="eq", name="eq")
                vec.tensor_tensor(out=EQ[:], in0=DN[:],
                                  in1=MINDN[:, sl].unsqueeze(2).broadcast_to(
                                      (128, G, 64)), op=OP.is_equal)
                vec.scalar_tensor_tensor(out=EQ[:], in0=EQ[:], scalar=0.0,
                                         in1=IOTAN.unsqueeze(1).broadcast_to(
                                             (128, G, 64)),
                                         op0=OP.add, op1=OP.mult)
                vec.tensor_reduce(out=NIXP[:, sl], in_=EQ[:], axis=AX.X,
                                  op=OP.max)
                MINI = wp.tile([128, G], mybir.dt.bfloat16, tag="mini", name="mini")
                vec.tensor_scalar(out=MINI[:], in0=NIXP[:, sl], scalar1=-1.0,
                                  scalar2=64.0, op0=OP.mult, op1=OP.add)
                OH = wp.tile([128, G, 64], mybir.dt.bfloat16, tag="oh", name="oh")
                vec.tensor_tensor(out=OH[:],
                                  in0=IOTA.unsqueeze(1).broadcast_to((128, G, 64)),
                                  in1=MINI[:].unsqueeze(2).broadcast_to(
                                      (128, G, 64)), op=OP.is_equal)
                # rt extraction on PE: transpose one-hot, then ohT.T @ PBS
                RTP = pp.tile([128, G, 4], F32, tag="rtp", name="rtp")
                for g in range(G):
                    OHT = pp.tile([64, 128], mybir.dt.bfloat16, tag="oht",
                                  name="oht")
                    nc.tensor.transpose(OHT[:], OH[:, g, :], IDT[:])
                    OHTS = wp.tile([64, 128], mybir.dt.bfloat16, tag="ohts",
                                   name="ohts")
                    act.copy(out=OHTS[:], in_=OHT[:])
                    for k in range(3):
                        nc.tensor.matmul(out=RTP[:, g, :], lhsT=OHTS[:],
                                         rhs=PBS3[:, k, :],
                                         start=(k == 0), stop=(k == 2))
                act.copy(out=XT[:, :, sl], in_=RTP[:].transpose([0, 2, 1]))

            # ---------------- epilogue: per-location [128, NT] --------------
            AGN = DY[:, 0, :]
            VAL = SL[:, 6, :]
            ISV = SL[:, 7, :]

            def lt(tag):
                return wp.tile([128, NT], F32, tag=tag, name=tag)

            HM = lt("hm")
            act.activation(out=HM[:], in_=MINWN[:], func=AF.Exp, scale=1.0)
            vec.scalar_tensor_tensor(out=HM[:], in0=HM[:], scalar=SIG_LO,
                                     in1=HM[:], op0=OP.is_ge, op1=OP.mult)
            NW = lt("nw")
            vec.tensor_scalar(out=NW[:], in0=HM[:], scalar1=-1.0, scalar2=1.0,
                              op0=OP.mult, op1=OP.add)
            vec.tensor_tensor(out=NW[:], in0=NW[:], in1=NW[:], op=OP.mult)
            vec.tensor_tensor(out=NW[:], in0=NW[:], in1=NW[:], op=OP.mult)
            PC = lt("pc")
            act.activation(out=PC[:], in_=AGN, func=AF.Sigmoid)
            vec.tensor_scalar(out=PC[:], in0=PC[:], scalar1=SIG_LO,
                              scalar2=SIG_HI, op0=OP.max, op1=OP.min)
            Q = lt("q")
            vec.tensor_scalar(out=Q[:], in0=PC[:], scalar1=-1.0, scalar2=1.0,
                              op0=OP.mult, op1=OP.add)
            act.activation(out=Q[:], in_=Q[:], func=AF.Ln)  # log(1-pred)
            P2 = lt("p2")
            vec.tensor_tensor(out=P2[:], in0=PC[:], in1=PC[:], op=OP.mult)
            T1 = lt("t1")
            vec.tensor_tensor(out=T1[:], in0=Q[:], in1=P2[:], op=OP.mult)
            vec.tensor_tensor(out=T1[:], in0=T1[:], in1=NW[:], op=OP.mult)
            GT = lt("gt")
            vec.tensor_scalar(out=GT[:], in0=PC[:], scalar1=IGNORE_HIGH_FP,
                              scalar2=None, op0=OP.is_lt)
            vec.tensor_tensor(out=T1[:], in0=T1[:], in1=GT[:], op=OP.mult)
            vec.tensor_tensor(out=T1[:], in0=T1[:], in1=VAL, op=OP.mult)
            NEGA = cp.tile([128, 1], F32)
            vec.tensor_reduce(out=NEGA[:], in_=T1[:], axis=AX.X, op=OP.add)
            # validity + rt
            VM = lt("vm")
            vec.tensor_scalar(out=VM[:], in0=MINDN[:], scalar1=-INF, scalar2=None,
                              op0=OP.is_gt)
            vec.tensor_tensor(out=VM[:], in0=VM[:], in1=VAL, op=OP.mult)
            REGC = cp.tile([128, 1], F32)
            vec.tensor_reduce(out=REGC[:], in_=VM[:], axis=AX.X, op=OP.add)
            RT = wp.tile([128, 4, NT], F32, tag="rt", name="rt")
            vec.scalar_tensor_tensor(out=RT[:, 0:2, :], in0=XT[:, 0:2, :],
                                     scalar=-1.0, in1=SL[:, 0:2, :],
                                     op0=OP.mult, op1=OP.add)
            vec.scalar_tensor_tensor(out=RT[:, 2:4, :], in0=XT[:, 2:4, :],
                                     scalar=-1.0, in1=SL[:, 4:6, :],
                                     op0=OP.mult, op1=OP.add)
            # RT = signed_grid - XT = (l, t, r, b) of argmin box
            vec.tensor_tensor(out=RT[:], in0=RT[:],
                              in1=ISV.unsqueeze(1).broadcast_to((128, 4, NT)),
                              op=OP.mult)
            # rtf = rt*vm + (1-vm)   (exact select; vm in {0,1})
            RTF = wp.tile([128, 4, NT], F32, tag="rtf", name="rtf")
            vec.tensor_tensor(out=RTF[:], in0=RT[:],
                              in1=VM[:].unsqueeze(1).broadcast_to((128, 4, NT)),
                              op=OP.mult)
            VMN = lt("vmn")
            vec.tensor_scalar(out=VMN[:], in0=VM[:], scalar1=-1.0, scalar2=1.0,
                              op0=OP.mult, op1=OP.add)
            vec.tensor_tensor(out=RTF[:], in0=RTF[:],
                              in1=VMN[:].unsqueeze(1).broadcast_to((128, 4, NT)),
                              op=OP.add)
            # giou(pred, rtf)
            pl, pt = DY[:, 1, :], DY[:, 2, :]
            pr, pb = DY[:, 3, :], DY[:, 4, :]
            tl, tt_ = RTF[:, 0, :], RTF[:, 1, :]
            tr, tb = RTF[:, 2, :], RTF[:, 3, :]
            TA, PA, WI, GW, HI, GH = (lt("ta"), lt("pa"), lt("wi"), lt("gw"),
                                      lt("hi"), lt("gh"))
            SA, SB = lt("sa"), lt("sb")
            vec.tensor_tensor(out=SA[:], in0=tl, in1=tr, op=OP.add)
            vec.tensor_tensor(out=SB[:], in0=tt_, in1=tb, op=OP.add)
            vec.tensor_tensor(out=TA[:], in0=SA[:], in1=SB[:], op=OP.mult)
            vec.tensor_tensor(out=SA[:], in0=pl, in1=pr, op=OP.add)
            vec.tensor_tensor(out=SB[:], in0=pt, in1=pb, op=OP.add)
            vec.tensor_tensor(out=PA[:], in0=SA[:], in1=SB[:], op=OP.mult)
            vec.tensor_tensor(out=SA[:], in0=pl, in1=tl, op=OP.min)
            vec.tensor_tensor(out=SB[:], in0=pr, in1=tr, op=OP.min)
            vec.tensor_tensor(out=WI[:], in0=SA[:], in1=SB[:], op=OP.add)
            vec.tensor_tensor(out=SA[:], in0=pl, in1=tl, op=OP.max)
            vec.tensor_tensor(out=SB[:], in0=pr, in1=tr, op=OP.max)
            vec.tensor_tensor(out=GW[:], in0=SA[:], in1=SB[:], op=OP.add)
            vec.tensor_tensor(out=SA[:], in0=pb, in1=tb, op=OP.min)
            vec.tensor_tensor(out=SB[:], in0=pt, in1=tt_, op=OP.min)
            vec.tensor_tensor(out=HI[:], in0=SA[:], in1=SB[:], op=OP.add)
            vec.tensor_tensor(out=SA[:], in0=pb, in1=tb, op=OP.max)
            vec.tensor_tensor(out=SB[:], in0=pt, in1=tt_, op=OP.max)
            vec.tensor_tensor(out=GH[:], in0=SA[:], in1=SB[:], op=OP.add)
            AC = lt("ac")
            vec.tensor_tensor(out=AC[:], in0=GW[:], in1=GH[:], op=OP.mult)
            vec.tensor_scalar(out=AC[:], in0=AC[:], scalar1=EPS_AC,
                              scalar2=None, op0=OP.add)
            INTER = lt("inter")
            vec.tensor_tensor(out=INTER[:], in0=WI[:], in1=HI[:], op=OP.mult)
            UN = lt("un")
            vec.tensor_tensor(out=UN[:], in0=TA[:], in1=PA[:], op=OP.add)
            vec.tensor_tensor(out=UN[:], in0=UN[:], in1=INTER[:], op=OP.subtract)
            vec.tensor_scalar(out=SA[:], in0=INTER[:], scalar1=1.0,
                              scalar2=None, op0=OP.add)
            vec.tensor_scalar(out=SB[:], in0=UN[:], scalar1=1.0,
                              scalar2=None, op0=OP.add)
            IOU = lt("iou")
            vec.reciprocal(out=SB[:], in_=SB[:])
            vec.tensor_tensor(out=IOU[:], in0=SA[:], in1=SB[:], op=OP.mult)
            vec.tensor_tensor(out=SA[:], in0=AC[:], in1=UN[:], op=OP.subtract)
            vec.reciprocal(out=SB[:], in_=AC[:])
            vec.tensor_tensor(out=SB[:], in0=SA[:], in1=SB[:], op=OP.mult)
            vec.tensor_tensor(out=IOU[:], in0=IOU[:], in1=SB[:], op=OP.subtract)
            vec.tensor_scalar(out=IOU[:], in0=IOU[:], scalar1=-1.0, scalar2=1.0,
                              op0=OP.mult, op1=OP.add)  # 1 - giou
            vec.tensor_tensor(out=IOU[:], in0=IOU[:], in1=VM[:], op=OP.mult)
            REGA = cp.tile([128, 1], F32)
            vec.tensor_reduce(out=REGA[:], in_=IOU[:], axis=AX.X, op=OP.add)

            # ---------------- partial reduction + allreduce -----------------
            PART = cp.tile([128, 8], F32)
            vec.memset(PART[:], 0.0)
            vec.tensor_copy(out=PART[:, 0:1], in_=REGA[:])
            vec.tensor_copy(out=PART[:, 1:2], in_=REGC[:])
            vec.tensor_copy(out=PART[:, 3:4], in_=NEGA[:])
            vec.tensor_copy(out=PART[0:NBOX, 2:3], in_=POSS[:])
            vec.tensor_copy(out=PART[0:NBOX, 4:5], in_=NPOS[:])
            ONES = cp.tile([128, 1], F32)
            vec.memset(ONES[:], 1.0)
            PS = pp.tile([1, 8], F32, bufs=1)
            nc.tensor.matmul(out=PS[:], lhsT=ONES[:], rhs=PART[:],
                             start=True, stop=True)
            PSB = cp.tile([1, 8], F32)
            vec.tensor_copy(out=PSB[:], in_=PS[:])
            if dbg:
                sync.dma_start(out=pdbg[:], in_=PSB[:])
                sync.dma_start(out=minddbg[:], in_=MINDN[:])
                sync.dma_start(out=minwdbg[:], in_=MINWN[:])
                sync.dma_start(out=xtdbg[:], in_=XT[:])
                sync.dma_start(out=posdbg[:], in_=POSF[:])
                sync.dma_start(out=gvdbg[:], in_=GV[:])
            RED = cp.tile([1, 8], F32)
            if with_cc:
                sync.dma_start(out=pdram[:], in_=PSB[:])
                gps.collective_compute(
                    "AllReduce", OP.add,
                    replica_groups=[list(range(N_CORES))],
                    ins=[pdram[:]], outs=[ccout[:]])
                sync.dma_start(out=RED[:], in_=ccout[:])
            else:
                vec.tensor_copy(out=RED[:], in_=PSB[:])

            # ---------------- finalize ---------------------------------------
            NPA = cp.tile([1, 1], F32, tag="npa", name="npa")
            vec.tensor_scalar(out=NPA[:], in0=RED[:, 4:5], scalar1=1.0,
                              scalar2=None, op0=OP.max)
            RGN = cp.tile([1, 1], F32, tag="rgn", name="rgn")
            vec.tensor_scalar(out=RGN[:], in0=RED[:, 1:2], scalar1=1.0,
                              scalar2=None, op0=OP.max)
            O3 = cp.tile([1, 3], F32, tag="o3", name="o3")
            vec.reciprocal(out=RGN[:], in_=RGN[:])
            vec.tensor_tensor(out=O3[:, 0:1], in0=RED[:, 0:1], in1=RGN[:],
                              op=OP.mult)
            vec.reciprocal(out=NPA[:], in_=NPA[:])
            SCL = cp.tile([1, 1], F32, tag="scl", name="scl")
            vec.tensor_scalar(out=SCL[:], in0=RED[:, 2:3], scalar1=-0.125,
                              scalar2=None, op0=OP.mult)
            vec.tensor_tensor(out=O3[:, 1:2], in0=SCL[:], in1=NPA[:], op=OP.mult)
            vec.tensor_scalar(out=SCL[:], in0=RED[:, 3:4], scalar1=-0.375,
                              scalar2=None, op0=OP.mult)
            vec.tensor_tensor(out=O3[:, 2:3], in0=SCL[:], in1=NPA[:], op=OP.mult)
            sync.dma_start(out=out[:], in_=O3[:])
    nc.compile()
    return nc


# ------------------------------ host wrapper -------------------------------

def make_in_maps(boxes, agn_hm_pred, reg_pred):
    boxes = np.ascontiguousarray(np.asarray(boxes, np.float32))
    agn = np.ascontiguousarray(np.asarray(agn_hm_pred, np.float32))
    rp = np.ascontiguousarray(np.asarray(reg_pred, np.float32))
    agnfull = np.ascontiguousarray(agn.reshape(M_TOT, 1))
    in_maps = []
    for c in range(N_CORES):
        b, h = c // 2, c % 2
        idx = _SHARD_IDX[(b, h)]
        dyn = np.zeros((128, 5, NT), np.float32)
        a = np.zeros(NPAD, np.float32)
        a[:NV] = agn[idx]
        dyn[:, 0, :] = _pack(a)
        r = np.zeros((NPAD, 4), np.float32)
        r[:NV] = rp[idx]
        for k in range(4):
            dyn[:, 1 + k, :] = _pack(np.ascontiguousarray(r[:, k]))
        corec = np.zeros((NBOX, 8), np.float32)
        for l in range(5):
            corec[:, l] = BASE[l] + b * LOC[l]
        corec[:, 5] = 1.0 if h == 0 else 0.0
        in_maps.append({
            "locst": _LOCSTAT[h],
            "dyn": np.ascontiguousarray(dyn),
            "boxesT": np.ascontiguousarray(boxes[b].T),
            "boxesP": np.ascontiguousarray(np.tile(boxes[b], (2, 1))),
            "agnfull": agnfull,
            "corec": corec,
            "miscc": _MISC,
        })
    return in_maps


_NC_CACHE = {}
LAST_RESULT = None


def _get_nc():
    if "nc" not in _NC_CACHE:
        _NC_CACHE["nc"] = build_nc(with_cc=True, dbg=False)
    return _NC_CACHE["nc"]


def kernel(boxes, gt_classes=None, agn_hm_pred=None, reg_pred=None):
    global LAST_RESULT
    in_maps = make_in_maps(boxes, agn_hm_pred, reg_pred)
    nc = _get_nc()
    res = run_bass_kernel_spmd(nc, in_maps, core_ids=list(range(N_CORES)))
    LAST_RESULT = res
    return np.asarray(res.results[0]["out"], np.float32)



# revision 15
# speedup vs baseline: 1.8255x; 1.8255x over previous
"""CenterNet loss (GT assignment + focal/giou losses) on 8 Trainium2 cores.

Sharding: core c handles image b = c//2 and half h = c%2 of EVERY FPN level
(so all 8 cores run an identical SPMD tile schedule). Each core produces
partial sums [1,8]: (giou_sum, reg_cnt, pos_sum, neg_sum, npos, ...); the
host sums the 8 partial vectors and computes the final 3-vector (the
"gather/unshard" step for scalar losses).

Device kernel (v2, fp16):
- pair loop in fp16: grid/cdis coords are integers (exact in fp16); the
  distance chain uses /32-scaled coordinates so squares stay in range.
- masks use additive -60000 penalties; d = -wdist2 + penalties, argmax.
- reg-target extraction: one-hot over boxes, transposed on the PE in
  [128,128] 2-tile blocks, then ONE fp16 2-split block-diagonal matmul
  per block extracts (x0,y0,-x1,-y1) of the argmin box per location.
"""

import numpy as np
import ml_dtypes

import concourse.bass as bass
import concourse.bacc as bacc
import concourse.tile as tile
from concourse import ap_utils, mybir
from concourse.bass_utils import run_bass_kernel_spmd

F32 = mybir.dt.float32
F16 = mybir.dt.float16
BF16 = mybir.dt.bfloat16
I32 = mybir.dt.int32
AF = mybir.ActivationFunctionType
OP = mybir.AluOpType
AX = mybir.AxisListType

# ---------------- problem constants (hardcoded from the nn.Module) ---------
B, NBOX = 4, 64
STRIDES = (8, 16, 32, 64, 128)
LEVEL_HW = ((128, 128), (64, 64), (32, 32), (16, 16), (8, 8))
SIZES = ((0.0, 80.0), (64.0, 160.0), (128.0, 320.0), (256.0, 640.0), (512.0, 1e7))
LOC = [h * w for h, w in LEVEL_HW]          # [16384, 4096, 1024, 256, 64]
M_IMG = sum(LOC)                            # 21824
M_TOT = B * M_IMG                           # 87296
BASE = [0, 65536, 81920, 86016, 87040]      # global level bases (level-major)
HALF = [m // 2 for m in LOC]                # per-core per-level loc counts
NT = 86                                     # 128-loc tiles per core
NV = sum(HALF)                              # 10912 valid locs per core
NPAD = NT * 128                             # 11008
MIN_RADIUS2 = 16.0
DELTA = (1 - 0.8) / (1 + 0.8)
K_R2 = float(np.float32(DELTA ** 2 * 2))    # radius2 = max(K_R2*area, 16)
SIG_LO = float(np.float32(1e-4))
SIG_HI = float(np.float32(1.0 - 1e-4))
EPS_AC = float(np.float32(1e-7))
IGNORE_HIGH_FP = 0.85
MAGIC = 8388608.0  # 2^23: u+MAGIC-MAGIC rounds u to nearest int (u < 2^22)
NEGK = -60000.0    # mask penalty (valid wdist2 is always < 2200)
CSC = 1.0 / 32.0   # distance-coordinate scale (keeps squares in fp16 range)
# supergroups: (tile0, n_tiles, level)
SG = [(0, 16, 0), (16, 16, 0), (32, 16, 0), (48, 16, 0),
      (64, 16, 1), (80, 4, 2), (84, 1, 3), (85, 1, 4)]

N_CORES = 8


def _pack(vec):
    """[NPAD] (loc j = t*128+p) -> [128, NT] with [p, t] layout."""
    return np.ascontiguousarray(vec.reshape(NT, 128).T)


def _grids_per_level():
    gs = []
    for (h, w), s in zip(LEVEL_HW, STRIDES):
        ys, xs = np.meshgrid(np.arange(h) * s, np.arange(w) * s, indexing="ij")
        g = np.stack([xs.reshape(-1), ys.reshape(-1)], 1).astype(np.float32) + s // 2
        gs.append(g)
    return gs


def _half_concat(per_level_fn, h):
    """Concat per-level arrays for half h, pad to NPAD."""
    parts = [per_level_fn(l, h) for l in range(5)]
    cat = np.concatenate(parts, 0)
    pad_shape = (NPAD - NV,) + cat.shape[1:]
    return np.concatenate([cat, np.zeros(pad_shape, cat.dtype)], 0)


_GRIDS = _grids_per_level()


def _build_locstat(h):
    """[128, 6, NT] f32: planes gx, gy, -gx, -gy, valid, inv_s."""
    g = _half_concat(lambda l, hh: _GRIDS[l][hh * HALF[l]:(hh + 1) * HALF[l]], h)
    gx, gy = g[:, 0], g[:, 1]
    valid = np.zeros(NPAD, np.float32)
    valid[:NV] = 1.0
    inv_s = _half_concat(
        lambda l, hh: np.full(HALF[l], 1.0 / STRIDES[l], np.float32), h)
    inv_s[NV:] = 1.0
    planes = [gx, gy, -gx, -gy, valid, inv_s]
    out = np.stack([_pack(p.astype(np.float32)) for p in planes], 1)
    return np.ascontiguousarray(out)  # [128, 6, NT]


def _build_locstat16(h):
    """[128, 4, NT] fp16: planes gx/32, gy/32, gx, gy."""
    g = _half_concat(lambda l, hh: _GRIDS[l][hh * HALF[l]:(hh + 1) * HALF[l]], h)
    gx, gy = g[:, 0], g[:, 1]
    planes = [gx * CSC, gy * CSC, gx, gy]
    out = np.stack([_pack(p.astype(np.float32)) for p in planes], 1)
    return np.ascontiguousarray(out.astype(np.float16))


_LOCSTAT = [_build_locstat(0), _build_locstat(1)]
_LOCSTAT16 = [_build_locstat16(0), _build_locstat16(1)]

# iota constants [128, 2, 64] bf16: plane0 = 0..63, plane1 = 64 - iota
_MISC = np.ascontiguousarray(np.stack([
    np.broadcast_to(np.arange(64, dtype=np.float32), (128, 64)),
    np.broadcast_to(64.0 - np.arange(64, dtype=np.float32), (128, 64)),
], 1)).astype(ml_dtypes.bfloat16)


def _shard_idx(b, h):
    """Global level-major indices of core (b, h)'s NV locations."""
    parts = [BASE[l] + b * LOC[l] + h * HALF[l] + np.arange(HALF[l])
             for l in range(5)]
    return np.concatenate(parts, 0)


_SHARD_IDX = {(b, h): _shard_idx(b, h) for b in range(B) for h in range(2)}


def _corec(b, h):
    """[NBOX, 28] f32 per-core consts for the pos part:
    cols 0-4 global level base (+image), 5 even-core gate, 6-10 1/s,
    11-15 level width, 16-20 lo, 21-25 hi."""
    c = np.zeros((NBOX, 28), np.float32)
    for l in range(5):
        c[:, l] = BASE[l] + b * LOC[l]
        c[:, 6 + l] = 1.0 / STRIDES[l]
        c[:, 11 + l] = LEVEL_HW[l][1]
        c[:, 16 + l] = SIZES[l][0]
        c[:, 21 + l] = SIZES[l][1]
    c[:, 5] = 1.0 if h == 0 else 0.0
    return c


# ------------------------------ device program -----------------------------

def build_nc(dbg=False):
    nc = bacc.Bacc(trn_type="TRN2", num_devices=N_CORES)
    locst = nc.dram_tensor("locst", [128, 6, NT], F32, kind="ExternalInput")
    locst16 = nc.dram_tensor("locst16", [128, 4, NT], F16, kind="ExternalInput")
    dyn = nc.dram_tensor("dyn", [128, 5, NT], F32, kind="ExternalInput")
    boxesT = nc.dram_tensor("boxesT", [4, NBOX], F32, kind="ExternalInput")
    boxesP = nc.dram_tensor("boxesP", [2 * NBOX, 4], F32, kind="ExternalInput")
    agnfull = nc.dram_tensor("agnfull", [M_TOT, 1], F32, kind="ExternalInput")
    corec = nc.dram_tensor("corec", [NBOX, 28], F32, kind="ExternalInput")
    miscc = nc.dram_tensor("miscc", [128, 2, 64], BF16, kind="ExternalInput")
    out = nc.dram_tensor("out", [1, 8], F32, kind="ExternalOutput")
    if dbg:
        minddbg = nc.dram_tensor("minddbg", [128, NT], F32, kind="ExternalOutput")
        minwdbg = nc.dram_tensor("minwdbg", [128, NT], F32, kind="ExternalOutput")
        xtdbg = nc.dram_tensor("xtdbg", [128, 4, NT], F32, kind="ExternalOutput")
        posdbg = nc.dram_tensor("posdbg", [NBOX, 5], F32, kind="ExternalOutput")
        gvdbg = nc.dram_tensor("gvdbg", [NBOX, 5], F32, kind="ExternalOutput")

    vec, act, gps, sync = nc.vector, nc.scalar, nc.gpsimd, nc.sync

    with tile.TileContext(nc) as tc:
        with tc.tile_pool(name="const", bufs=1) as cp, \
             tc.tile_pool(name="work", bufs=3) as wp, \
             tc.tile_pool(name="ppool", bufs=2, space="PSUM") as pp:

            # ---------------- loads ----------------
            SL = cp.tile([128, 6, NT], F32)
            sync.dma_start(out=SL[:], in_=locst[:])
            SL16 = cp.tile([128, 4, NT], F16)
            sync.dma_start(out=SL16[:], in_=locst16[:])
            DY = cp.tile([128, 5, NT], F32)
            sync.dma_start(out=DY[:], in_=dyn[:])
            BBh = boxesT[:]
            BB = cp.tile([128, 4, NBOX], F32)
            bt_bc = bass.AP(tensor=BBh.tensor, offset=BBh.offset,
                            ap=[[0, 128], [NBOX, 4], [1, NBOX]])
            sync.dma_start(out=BB[:], in_=bt_bc)
            BP = cp.tile([2 * NBOX, 4], F32)
            sync.dma_start(out=BP[:], in_=boxesP[:])
            CO = cp.tile([NBOX, 28], F32)
            sync.dma_start(out=CO[:], in_=corec[:])
            MI = cp.tile([128, 2, 64], BF16)
            sync.dma_start(out=MI[:], in_=miscc[:])
            IOTA = MI[:, 0, :]
            IOTAN = MI[:, 1, :]

            # ---------------- per-box precompute ([128, 64] broadcast) -----
            from concourse.masks import make_identity
            IDT = cp.tile([128, 128], BF16)
            make_identity(nc, IDT[:])
            x0, y0 = BB[:, 0, :], BB[:, 1, :]
            x1, y1 = BB[:, 2, :], BB[:, 3, :]
            CXY = cp.tile([128, 2, NBOX], F32)
            CX, CY = CXY[:, 0, :], CXY[:, 1, :]
            vec.tensor_tensor(out=CXY[:], in0=BB[:, 0:2, :], in1=BB[:, 2:4, :],
                              op=OP.add)
            vec.tensor_scalar(out=CXY[:], in0=CXY[:], scalar1=0.5, scalar2=None,
                              op0=OP.mult)
            # W2C = (w/2, h/2) per box;  IW2 = 32/w2 (fp16, scaled units)
            W2C = cp.tile([128, 2, NBOX], F32)
            vec.tensor_tensor(out=W2C[:], in0=BB[:, 2:4, :], in1=BB[:, 0:2, :],
                              op=OP.subtract)
            vec.tensor_scalar(out=W2C[:], in0=W2C[:], scalar1=0.5, scalar2=None,
                              op0=OP.mult)
            W2SF = cp.tile([128, 2, NBOX], F32)
            vec.tensor_scalar(out=W2SF[:], in0=W2C[:], scalar1=CSC,
                              scalar2=None, op0=OP.mult)
            W2S2 = cp.tile([128, 2, NBOX], F16)
            vec.tensor_tensor(out=W2S2[:], in0=W2SF[:], in1=W2SF[:],
                              op=OP.mult)
            S1 = cp.tile([128, NBOX], F32)
            S2 = cp.tile([128, NBOX], F32)
            S3 = cp.tile([128, NBOX], F32)
            # radius2 = max(K_R2 * area, 16);  IR2N = -1024/radius2 (fp16)
            R2 = cp.tile([128, NBOX], F32)
            vec.tensor_tensor(out=S1[:], in0=x1, in1=x0, op=OP.subtract)  # w
            vec.tensor_tensor(out=S2[:], in0=y1, in1=y0, op=OP.subtract)  # h
            vec.tensor_tensor(out=R2[:], in0=S1[:], in1=S2[:], op=OP.mult)
            vec.tensor_scalar(out=R2[:], in0=R2[:], scalar1=K_R2,
                              scalar2=MIN_RADIUS2, op0=OP.mult, op1=OP.max)
            IR2F = cp.tile([128, NBOX], F32)
            vec.reciprocal(out=IR2F[:], in_=R2[:])
            IR2N = cp.tile([128, NBOX], F16)
            vec.tensor_scalar(out=IR2N[:], in0=IR2F[:], scalar1=-1024.0,
                              scalar2=None, op0=OP.mult)
            # crit = sqrt(w^2 + h^2) / 2  (per box)
            CRIT = cp.tile([128, NBOX], F32)
            vec.tensor_tensor(out=S1[:], in0=S1[:], in1=S1[:], op=OP.mult)
            vec.tensor_tensor(out=S2[:], in0=S2[:], in1=S2[:], op=OP.mult)
            vec.tensor_tensor(out=S1[:], in0=S1[:], in1=S2[:], op=OP.add)
            act.activation(out=CRIT[:], in_=S1[:], func=AF.Sqrt, scale=0.25)
            # PBS [128, 4] = (x0, y0, -x1, -y1), both 64-box halves identical
            PBS = cp.tile([2 * NBOX, 4], F32)
            vec.tensor_copy(out=PBS[:, 0:2], in_=BP[:, 0:2])
            vec.tensor_scalar(out=PBS[:, 2:4], in0=BP[:, 2:4], scalar1=-1.0,
                              scalar2=None, op0=OP.mult)
            # bf16 2-split of PBS into block-diagonal rhs PBSD [128, 16]:
            # rows 0:64 -> cols 0:4 (hi), 8:12 (lo); rows 64:128 -> 4:8, 12:16
            PBSH = cp.tile([2 * NBOX, 4], BF16)
            vec.tensor_copy(out=PBSH[:], in_=PBS[:])
            PBSR = cp.tile([2 * NBOX, 4], F32)
            vec.tensor_copy(out=PBSR[:], in_=PBSH[:])
            PBSL = cp.tile([2 * NBOX, 4], BF16)
            vec.tensor_tensor(out=PBSL[:], in0=PBS[:], in1=PBSR[:],
                              op=OP.subtract)
            PBSD = cp.tile([128, 16], BF16)
            vec.memset(PBSD[:], 0.0)
            vec.tensor_copy(out=PBSD[0:NBOX, 0:4], in_=PBSH[0:NBOX, :])
            vec.tensor_copy(out=PBSD[NBOX:128, 4:8], in_=PBSH[NBOX:128, :])
            vec.tensor_copy(out=PBSD[0:NBOX, 8:12], in_=PBSL[0:NBOX, :])
            vec.tensor_copy(out=PBSD[NBOX:128, 12:16], in_=PBSL[NBOX:128, :])
            # per-level consts: BCNN16[l] = carednot penalty {0,-60000(x2)};
            # CB4_16[l] = (cx/32, cy/32, cdisx, cdisy) fp16
            BCNN, CB4 = [], []
            for l in range(5):
                s = float(STRIDES[l])
                lo, hi = SIZES[l]
                vec.tensor_scalar(out=S1[:], in0=CRIT[:], scalar1=float(lo),
                                  scalar2=NEGK, op0=OP.is_lt, op1=OP.mult)
                vec.tensor_scalar(out=S2[:], in0=CRIT[:], scalar1=float(hi),
                                  scalar2=NEGK, op0=OP.is_gt, op1=OP.mult)
                cn = cp.tile([128, NBOX], F16, tag=f"bcnn{l}", name=f"bcnn{l}")
                vec.tensor_tensor(out=cn[:], in0=S1[:], in1=S2[:], op=OP.add)
                BCNN.append(cn)
                t = cp.tile([128, 4, NBOX], F16, tag=f"cb4{l}", name=f"cb4{l}")
                vec.tensor_scalar(out=t[:, 0:2, :], in0=CXY[:], scalar1=CSC,
                                  scalar2=None, op0=OP.mult)
                for ci, src_ in enumerate((CX, CY)):
                    vec.tensor_scalar(out=S1[:], in0=src_, scalar1=1.0 / s,
                                      scalar2=None, op0=OP.mult)  # u = c/s
                    vec.tensor_scalar(out=S2[:], in0=S1[:], scalar1=MAGIC,
                                      scalar2=None, op0=OP.add)
                    vec.tensor_scalar(out=S2[:], in0=S2[:], scalar1=-MAGIC,
                                      scalar2=None, op0=OP.add)
                    vec.tensor_tensor(out=S3[:], in0=S2[:], in1=S1[:], op=OP.is_gt)
                    vec.tensor_tensor(out=S2[:], in0=S2[:], in1=S3[:],
                                      op=OP.subtract)  # floor(c/s)
                    vec.tensor_scalar(out=t[:, 2 + ci, :], in0=S2[:], scalar1=s,
                                      scalar2=s / 2.0, op0=OP.mult, op1=OP.add)
                CB4.append(t)

            # ---------------- pos part (boxes on partitions, [64, 5]) -------
            cx = cp.tile([NBOX, 1], F32, tag="pcx", name="pcx")
            cy = cp.tile([NBOX, 1], F32, tag="pcy", name="pcy")
            vec.tensor_tensor(out=cx[:], in0=BP[0:NBOX, 0:1], in1=BP[0:NBOX, 2:3], op=OP.add)
            vec.tensor_scalar(out=cx[:], in0=cx[:], scalar1=0.5, scalar2=None,
                              op0=OP.mult)
            vec.tensor_tensor(out=cy[:], in0=BP[0:NBOX, 1:2], in1=BP[0:NBOX, 3:4], op=OP.add)
            vec.tensor_scalar(out=cy[:], in0=cy[:], scalar1=0.5, scalar2=None,
                              op0=OP.mult)
            pw = cp.tile([NBOX, 1], F32, tag="ppw", name="ppw")
            ph = cp.tile([NBOX, 1], F32, tag="pph", name="pph")
            vec.tensor_tensor(out=pw[:], in0=BP[0:NBOX, 2:3], in1=BP[0:NBOX, 0:1],
                              op=OP.subtract)
            vec.tensor_tensor(out=ph[:], in0=BP[0:NBOX, 3:4], in1=BP[0:NBOX, 1:2],
                              op=OP.subtract)
            vec.tensor_tensor(out=pw[:], in0=pw[:], in1=pw[:], op=OP.mult)
            vec.tensor_tensor(out=ph[:], in0=ph[:], in1=ph[:], op=OP.mult)
            vec.tensor_tensor(out=pw[:], in0=pw[:], in1=ph[:], op=OP.add)
            pcrit = cp.tile([NBOX, 1], F32, tag="pcrit", name="pcrit")
            act.activation(out=pcrit[:], in_=pw[:], func=AF.Sqrt, scale=0.25)

            def p5(tag):
                return cp.tile([NBOX, 5], F32, tag=tag, name=tag)

            PA5, PB5, PC5, PD5, PE5 = (p5("pa5"), p5("pb5"), p5("pc5"),
                                       p5("pd5"), p5("pe5"))
            ISL, WLV = CO[:, 6:11], CO[:, 11:16]
            LOV, HIV = CO[:, 16:21], CO[:, 21:26]
            # ci_x = floor(cx/s) for all 5 levels at once
            vec.tensor_tensor(out=PD5[:], in0=cx[:].broadcast_to((NBOX, 5)),
                              in1=ISL, op=OP.mult)
            vec.tensor_scalar(out=PA5[:], in0=PD5[:], scalar1=MAGIC,
                              scalar2=-MAGIC, op0=OP.add, op1=OP.add)
            vec.tensor_tensor(out=PE5[:], in0=PA5[:], in1=PD5[:], op=OP.is_gt)
            vec.tensor_tensor(out=PA5[:], in0=PA5[:], in1=PE5[:], op=OP.subtract)
            # ci_y
            vec.tensor_tensor(out=PD5[:], in0=cy[:].broadcast_to((NBOX, 5)),
                              in1=ISL, op=OP.mult)
            vec.tensor_scalar(out=PB5[:], in0=PD5[:], scalar1=MAGIC,
                              scalar2=-MAGIC, op0=OP.add, op1=OP.add)
            vec.tensor_tensor(out=PE5[:], in0=PB5[:], in1=PD5[:], op=OP.is_gt)
            vec.tensor_tensor(out=PB5[:], in0=PB5[:], in1=PE5[:], op=OP.subtract)
            # pos = base + ci_y*W + ci_x, clamped
            vec.tensor_tensor(out=PC5[:], in0=PB5[:], in1=WLV, op=OP.mult)
            vec.tensor_tensor(out=PC5[:], in0=PC5[:], in1=PA5[:], op=OP.add)
            vec.tensor_tensor(out=PC5[:], in0=PC5[:], in1=CO[:, 0:5], op=OP.add)
            POSF = p5("posf")
            vec.tensor_scalar(out=POSF[:], in0=PC5[:], scalar1=0.0,
                              scalar2=float(M_TOT - 1), op0=OP.max, op1=OP.min)
            # pos mask
            PM = p5("pm")
            vec.tensor_tensor(out=PA5[:], in0=pcrit[:].broadcast_to((NBOX, 5)),
                              in1=LOV, op=OP.is_ge)
            vec.tensor_tensor(out=PB5[:], in0=pcrit[:].broadcast_to((NBOX, 5)),
                              in1=HIV, op=OP.is_le)
            vec.tensor_tensor(out=PM[:], in0=PA5[:], in1=PB5[:], op=OP.mult)
            POSI = cp.tile([NBOX, 5], I32, tag="posi", name="posi")
            vec.tensor_copy(out=POSI[:], in_=POSF[:])
            GV = cp.tile([NBOX, 5], F32, tag="gv", name="gv")
            for l in range(5):
                gps.indirect_dma_start(
                    out=GV[:, l:l + 1], out_offset=None, in_=agnfull[:],
                    in_offset=bass.IndirectOffsetOnAxis(ap=POSI[:, l:l + 1],
                                                        axis=0))
            PPRED = cp.tile([NBOX, 5], F32, tag="ppred", name="ppred")
            act.activation(out=PPRED[:], in_=GV[:], func=AF.Sigmoid)
            vec.tensor_scalar(out=PPRED[:], in0=PPRED[:], scalar1=SIG_LO,
                              scalar2=SIG_HI, op0=OP.max, op1=OP.min)
            QQ = cp.tile([NBOX, 5], F32, tag="qq", name="qq")
            vec.tensor_scalar(out=QQ[:], in0=PPRED[:], scalar1=-1.0, scalar2=1.0,
                              op0=OP.mult, op1=OP.add)
            vec.tensor_tensor(out=QQ[:], in0=QQ[:], in1=QQ[:], op=OP.mult)
            LGP = cp.tile([NBOX, 5], F32, tag="lgp", name="lgp")
            act.activation(out=LGP[:], in_=PPRED[:], func=AF.Ln)
            vec.tensor_tensor(out=LGP[:], in0=LGP[:], in1=QQ[:], op=OP.mult)
            vec.tensor_tensor(out=LGP[:], in0=LGP[:], in1=PM[:], op=OP.mult)
            # gate odd cores to zero (pos part owned by even core of each image)
            vec.tensor_scalar(out=LGP[:], in0=LGP[:], scalar1=CO[:, 5:6],
                              scalar2=None, op0=OP.mult)
            vec.tensor_scalar(out=PM[:], in0=PM[:], scalar1=CO[:, 5:6],
                              scalar2=None, op0=OP.mult)
            POSS = cp.tile([NBOX, 1], F32, tag="poss", name="poss")
            vec.tensor_reduce(out=POSS[:], in_=LGP[:], axis=AX.X, op=OP.add)
            NPOS = cp.tile([NBOX, 1], F32, tag="npos", name="npos")
            vec.tensor_reduce(out=NPOS[:], in_=PM[:], axis=AX.X, op=OP.add)

            # ---------------- main pair loop (fp16) -------------------------
            # negated convention: MINWN = -min(wdist2) = max(-wdist2),
            # MINDN = max(DN) where DN = -wdist2 + penalties,
            # NIXP = 64 - argmin(d).
            MINWN = cp.tile([128, NT], F16)
            MINDN = cp.tile([128, NT], F16)
            NIXP = cp.tile([128, NT], BF16)
            MINI = cp.tile([128, NT], BF16)
            OH2 = cp.tile([128, 2, 64], BF16)     # one-hot for tiles 84+85
            XT = cp.tile([128, 4, NT], F32)       # selected (x0, y0, -x1, -y1)

            def rt_blocks(oh_tile, lb0, gt0, nblk):
                """Extract reg targets for `nblk` 2-tile blocks.

                oh_tile: one-hot tile [128, *, 64]; local tile lb0 onward.
                gt0: global tile index of oh_tile[:, lb0]."""
                for g4 in range(0, nblk, 4):
                    blks = range(g4, min(g4 + 4, nblk))
                    nb = len(blks)
                    RTP4 = pp.tile([128, 64], F32, tag="rtp4", name="rtp4")
                    for j, blk in enumerate(blks):
                        OHT = pp.tile([128, 128], BF16, tag="oht", name="oht")
                        base = oh_tile[:, lb0 + 2 * blk, :]
                        oh_blk = bass.AP(
                            tensor=base.tensor, offset=base.offset,
                            ap=[list(base.ap[0]), [1, 128]])
                        nc.tensor.transpose(OHT[:], oh_blk, IDT[:])
                        OHS = wp.tile([128, 128], BF16, tag="ohs", name="ohs")
                        act.copy(out=OHS[:], in_=OHT[:])
                        nc.tensor.matmul(out=RTP4[:, 16 * j:16 * (j + 1)],
                                         lhsT=OHS[:], rhs=PBSD[:],
                                         start=True, stop=True)
                    # combine hi+lo splits: XT[:, c, gt0+2blk+ab] = hi + lo
                    RTS = wp.tile([128, 64], F32, tag="rts", name="rts")
                    act.copy(out=RTS[:], in_=RTP4[:])
                    t0_ = gt0 + 2 * g4
                    xt_base = XT[:, :, t0_:t0_ + 2 * nb]
                    xt_out = bass.AP(
                        tensor=xt_base.tensor, offset=xt_base.offset,
                        ap=[list(xt_base.ap[0]), [NT, 4], [2, nb], [1, 2]])
                    hi_base = RTS[:]
                    hi_in = bass.AP(
                        tensor=hi_base.tensor, offset=hi_base.offset,
                        ap=[list(hi_base.ap[0]), [1, 4], [16, nb], [4, 2]])
                    lo_base = RTS[:, 8:]
                    lo_in = bass.AP(
                        tensor=lo_base.tensor, offset=lo_base.offset,
                        ap=[list(lo_base.ap[0]), [1, 4], [16, nb], [4, 2]])
                    vec.tensor_tensor(out=xt_out, in0=hi_in, in1=lo_in,
                                      op=OP.add)

            for (t0, G, l) in SG:
                s = float(STRIDES[l])
                sl = slice(t0, t0 + G)

                def bb1(t2d, n=1):  # [128,n,64] const -> [128,n,G,64]
                    if n == 1:
                        return t2d.unsqueeze(1).broadcast_to((128, G, 64))
                    return t2d.unsqueeze(2).broadcast_to((128, n, G, 64))

                # DF = (dx', dy', dxc, dyc): scaled center diffs + cdis diffs
                DF = wp.tile([128, 4, G, 64], F16, tag="df", name="df")
                vec.tensor_tensor(
                    out=DF[:],
                    in0=SL16[:, 0:4, sl].unsqueeze(3).broadcast_to((128, 4, G, 64)),
                    in1=bb1(CB4[l][:], 4), op=OP.subtract)
                # squares of all 4 diff planes (act engine)
                SQ = wp.tile([128, 4, G, 64], F16, tag="sq", name="sq")
                act.square(out=SQ[:], in_=DF[:])
                # in-box test: dx'^2 >= (w2/32)^2 (either axis) -> penalty
                C2 = wp.tile([128, 2, G, 64], F16, tag="c2", name="c2")
                vec.tensor_tensor(out=C2[:], in0=SQ[:, 0:2],
                                  in1=bb1(W2S2[:], 2), op=OP.is_ge)
                PIN = wp.tile([128, G, 64], F16, tag="pin", name="pin")
                vec.tensor_tensor(out=PIN[:], in0=C2[:, 0], in1=C2[:, 1],
                                  op=OP.max)
                # 3x3 test: max(dxc^2, dyc^2) > s^2 -> penalty; == 0 -> peak
                M = wp.tile([128, G, 64], F16, tag="m", name="m")
                vec.tensor_tensor(out=M[:], in0=SQ[:, 2], in1=SQ[:, 3],
                                  op=OP.max)
                P3 = wp.tile([128, G, 64], F16, tag="p3", name="p3")
                vec.tensor_scalar(out=P3[:], in0=M[:], scalar1=s * s,
                                  scalar2=NEGK, op0=OP.is_gt, op1=OP.mult)
                SP = wp.tile([128, G, 64], F16, tag="sp", name="sp")
                vec.scalar_tensor_tensor(out=SP[:], in0=PIN[:], scalar=NEGK,
                                         in1=P3[:], op0=OP.mult, op1=OP.add)
                # dist2 (scaled) with peak zeroing, then * (-1024/r2)
                D2 = wp.tile([128, G, 64], F16, tag="d2", name="d2")
                vec.tensor_tensor(out=D2[:], in0=SQ[:, 0], in1=SQ[:, 1],
                                  op=OP.add)
                DZ = wp.tile([128, G, 64], F16, tag="dz", name="dz")
                vec.scalar_tensor_tensor(out=DZ[:], in0=M[:], scalar=0.0,
                                         in1=D2[:], op0=OP.not_equal, op1=OP.mult)
                WDN = wp.tile([128, G, 64], F16, tag="wdn", name="wdn")
                vec.tensor_tensor(out=WDN[:], in0=DZ[:], in1=bb1(IR2N[:]),
                                  op=OP.mult)  # -wdist2
                vec.tensor_reduce(out=MINWN[:, sl], in_=WDN[:], axis=AX.X,
                                  op=OP.max)
                T9 = wp.tile([128, G, 64], F16, tag="t9", name="t9")
                vec.tensor_tensor(out=T9[:], in0=WDN[:], in1=SP[:], op=OP.add)
                DN = wp.tile([128, G, 64], F16, tag="dn", name="dn")
                vec.tensor_tensor(out=DN[:], in0=T9[:], in1=bb1(BCNN[l][:]),
                                  op=OP.add)
                vec.tensor_reduce(out=MINDN[:, sl], in_=DN[:], axis=AX.X,
                                  op=OP.max)
                EQ = wp.tile([128, G, 64], BF16, tag="eq", name="eq")
                vec.tensor_tensor(out=EQ[:], in0=DN[:],
                                  in1=MINDN[:, sl].unsqueeze(2).broadcast_to(
                                      (128, G, 64)), op=OP.is_equal)
                EQN = wp.tile([128, G, 64], BF16, tag="eqn", name="eqn")
                vec.tensor_tensor(out=EQN[:], in0=EQ[:],
                                  in1=IOTAN.unsqueeze(1).broadcast_to(
                                      (128, G, 64)), op=OP.mult)
                vec.tensor_reduce(out=NIXP[:, sl], in_=EQN[:], axis=AX.X,
                                  op=OP.max)
                vec.tensor_scalar(out=MINI[:, sl], in0=NIXP[:, sl], scalar1=-1.0,
                                  scalar2=64.0, op0=OP.mult, op1=OP.add)
                if G > 1:
                    OH = wp.tile([128, G, 64], BF16, tag="oh", name="oh")
                    oh_dst, lb0 = OH[:], 0
                else:
                    oh_dst, lb0 = OH2[:, t0 - 84:t0 - 83, :], 0
                vec.tensor_tensor(out=oh_dst,
                                  in0=IOTA.unsqueeze(1).broadcast_to((128, G, 64)),
                                  in1=MINI[:, sl].unsqueeze(2).broadcast_to(
                                      (128, G, 64)), op=OP.is_equal)
                if G > 1:
                    rt_blocks(OH, 0, t0, G // 2)

            # tiles 84 (L3) + 85 (L4) share one 2-tile block
            rt_blocks(OH2, 0, 84, 1)

            # ---------------- epilogue: per-location [128, NT] --------------
            AGN = DY[:, 0, :]
            VAL = SL[:, 4, :]
            ISV = SL[:, 5, :]

            def lt(tag):
                return wp.tile([128, NT], F32, tag=tag, name=tag)

            HM = lt("hm")
            act.activation(out=HM[:], in_=MINWN[:], func=AF.Exp, scale=1.0)
            vec.scalar_tensor_tensor(out=HM[:], in0=HM[:], scalar=SIG_LO,
                                     in1=HM[:], op0=OP.is_ge, op1=OP.mult)
            NW = lt("nw")
            vec.tensor_scalar(out=NW[:], in0=HM[:], scalar1=-1.0, scalar2=1.0,
                              op0=OP.mult, op1=OP.add)
            vec.tensor_tensor(out=NW[:], in0=NW[:], in1=NW[:], op=OP.mult)
            vec.tensor_tensor(out=NW[:], in0=NW[:], in1=NW[:], op=OP.mult)
            PC = lt("pc")
            act.activation(out=PC[:], in_=AGN, func=AF.Sigmoid)
            vec.tensor_scalar(out=PC[:], in0=PC[:], scalar1=SIG_LO,
                              scalar2=SIG_HI, op0=OP.max, op1=OP.min)
            Q = lt("q")
            vec.tensor_scalar(out=Q[:], in0=PC[:], scalar1=-1.0, scalar2=1.0,
                              op0=OP.mult, op1=OP.add)
            act.activation(out=Q[:], in_=Q[:], func=AF.Ln)  # log(1-pred)
            P2 = lt("p2")
            vec.tensor_tensor(out=P2[:], in0=PC[:], in1=PC[:], op=OP.mult)
            T1 = lt("t1")
            vec.tensor_tensor(out=T1[:], in0=Q[:], in1=P2[:], op=OP.mult)
            vec.tensor_tensor(out=T1[:], in0=T1[:], in1=NW[:], op=OP.mult)
            GT = lt("gt")
            vec.tensor_scalar(out=GT[:], in0=PC[:], scalar1=IGNORE_HIGH_FP,
                              scalar2=None, op0=OP.is_lt)
            vec.tensor_tensor(out=T1[:], in0=T1[:], in1=GT[:], op=OP.mult)
            vec.tensor_tensor(out=T1[:], in0=T1[:], in1=VAL, op=OP.mult)
            NEGA = cp.tile([128, 1], F32)
            vec.tensor_reduce(out=NEGA[:], in_=T1[:], axis=AX.X, op=OP.add)
            # validity + rt
            VM = lt("vm")
            vec.tensor_scalar(out=VM[:], in0=MINDN[:], scalar1=-50000.0,
                              scalar2=None, op0=OP.is_gt)
            vec.tensor_tensor(out=VM[:], in0=VM[:], in1=VAL, op=OP.mult)
            REGC = cp.tile([128, 1], F32)
            vec.tensor_reduce(out=REGC[:], in_=VM[:], axis=AX.X, op=OP.add)
            RT = wp.tile([128, 4, NT], F32, tag="rt", name="rt")
            vec.scalar_tensor_tensor(out=RT[:, 0:2, :], in0=XT[:, 0:2, :],
                                     scalar=-1.0, in1=SL[:, 0:2, :],
                                     op0=OP.mult, op1=OP.add)
            vec.scalar_tensor_tensor(out=RT[:, 2:4, :], in0=XT[:, 2:4, :],
                                     scalar=-1.0, in1=SL[:, 2:4, :],
                                     op0=OP.mult, op1=OP.add)
            # RT = signed_grid - XT = (l, t, r, b) of argmin box; / stride
            vec.tensor_tensor(out=RT[:], in0=RT[:],
                              in1=ISV.unsqueeze(1).broadcast_to((128, 4, NT)),
                              op=OP.mult)
            # rtf = rt*vm + (1-vm)   (exact select; vm in {0,1})
            RTF = wp.tile([128, 4, NT], F32, tag="rtf", name="rtf")
            vec.tensor_tensor(out=RTF[:], in0=RT[:],
                              in1=VM[:].unsqueeze(1).broadcast_to((128, 4, NT)),
                              op=OP.mult)
            VMN = lt("vmn")
            vec.tensor_scalar(out=VMN[:], in0=VM[:], scalar1=-1.0, scalar2=1.0,
                              op0=OP.mult, op1=OP.add)
            vec.tensor_tensor(out=RTF[:], in0=RTF[:],
                              in1=VMN[:].unsqueeze(1).broadcast_to((128, 4, NT)),
                              op=OP.add)
            # giou(pred, rtf)
            pl, pt = DY[:, 1, :], DY[:, 2, :]
            pr, pb = DY[:, 3, :], DY[:, 4, :]
            tl, tt_ = RTF[:, 0, :], RTF[:, 1, :]
            tr, tb = RTF[:, 2, :], RTF[:, 3, :]
            TA, PA, WI, GW, HI, GH = (lt("ta"), lt("pa"), lt("wi"), lt("gw"),
                                      lt("hi"), lt("gh"))
            SA, SB = lt("sa"), lt("sb")
            vec.tensor_tensor(out=SA[:], in0=tl, in1=tr, op=OP.add)
            vec.tensor_tensor(out=SB[:], in0=tt_, in1=tb, op=OP.add)
            vec.tensor_tensor(out=TA[:], in0=SA[:], in1=SB[:], op=OP.mult)
            vec.tensor_tensor(out=SA[:], in0=pl, in1=pr, op=OP.add)
            vec.tensor_tensor(out=SB[:], in0=pt, in1=pb, op=OP.add)
            vec.tensor_tensor(out=PA[:], in0=SA[:], in1=SB[:], op=OP.mult)
            vec.tensor_tensor(out=SA[:], in0=pl, in1=tl, op=OP.min)
            vec.tensor_tensor(out=SB[:], in0=pr, in1=tr, op=OP.min)
            vec.tensor_tensor(out=WI[:], in0=SA[:], in1=SB[:], op=OP.add)
            vec.tensor_tensor(out=SA[:], in0=pl, in1=tl, op=OP.max)
            vec.tensor_tensor(out=SB[:], in0=pr, in1=tr, op=OP.max)
            vec.tensor_tensor(out=GW[:], in0=SA[:], in1=SB[:], op=OP.add)
            vec.tensor_tensor(out=SA[:], in0=pb, in1=tb, op=OP.min)
            vec.tensor_tensor(out=SB[:], in0=pt, in1=tt_, op=OP.min)
            vec.tensor_tensor(out=HI[:], in0=SA[:], in1=SB[:], op=OP.add)
            vec.tensor_tensor(out=SA[:], in0=pb, in1=tb, op=OP.max)
            vec.tensor_tensor(out=SB[:], in0=pt, in1=tt_, op=OP.max)
            vec.tensor_tensor(out=GH[:], in0=SA[:], in1=SB[:], op=OP.add)
            AC = lt("ac")
            vec.tensor_tensor(out=AC[:], in0=GW[:], in1=GH[:], op=OP.mult)
            vec.tensor_scalar(out=AC[:], in0=AC[:], scalar1=EPS_AC,
                              scalar2=None, op0=OP.add)
            INTER = lt("inter")
            vec.tensor_tensor(out=INTER[:], in0=WI[:], in1=HI[:], op=OP.mult)
            UN = lt("un")
            vec.tensor_tensor(out=UN[:], in0=TA[:], in1=PA[:], op=OP.add)
            vec.tensor_tensor(out=UN[:], in0=UN[:], in1=INTER[:], op=OP.subtract)
            vec.tensor_scalar(out=SA[:], in0=INTER[:], scalar1=1.0,
                              scalar2=None, op0=OP.add)
            vec.tensor_scalar(out=SB[:], in0=UN[:], scalar1=1.0,
                              scalar2=None, op0=OP.add)
            IOU = lt("iou")
            vec.reciprocal(out=SB[:], in_=SB[:])
            vec.tensor_tensor(out=IOU[:], in0=SA[:], in1=SB[:], op=OP.mult)
            vec.tensor_tensor(out=SA[:], in0=AC[:], in1=UN[:], op=OP.subtract)
            vec.reciprocal(out=SB[:], in_=AC[:])
            vec.tensor_tensor(out=SB[:], in0=SA[:], in1=SB[:], op=OP.mult)
            vec.tensor_tensor(out=IOU[:], in0=IOU[:], in1=SB[:], op=OP.subtract)
            vec.tensor_scalar(out=IOU[:], in0=IOU[:], scalar1=-1.0, scalar2=1.0,
                              op0=OP.mult, op1=OP.add)  # 1 - giou
            vec.tensor_tensor(out=IOU[:], in0=IOU[:], in1=VM[:], op=OP.mult)
            REGA = cp.tile([128, 1], F32)
            vec.tensor_reduce(out=REGA[:], in_=IOU[:], axis=AX.X, op=OP.add)

            # ---------------- partial reduction + output --------------------
            PART = cp.tile([128, 8], F32)
            vec.memset(PART[:], 0.0)
            vec.tensor_copy(out=PART[:, 0:1], in_=REGA[:])
            vec.tensor_copy(out=PART[:, 1:2], in_=REGC[:])
            vec.tensor_copy(out=PART[:, 3:4], in_=NEGA[:])
            vec.tensor_copy(out=PART[0:NBOX, 2:3], in_=POSS[:])
            vec.tensor_copy(out=PART[0:NBOX, 4:5], in_=NPOS[:])
            ONES = cp.tile([128, 1], F32)
            vec.memset(ONES[:], 1.0)
            PS = pp.tile([1, 8], F32, bufs=1)
            nc.tensor.matmul(out=PS[:], lhsT=ONES[:], rhs=PART[:],
                             start=True, stop=True)
            PSB = cp.tile([1, 8], F32)
            vec.tensor_copy(out=PSB[:], in_=PS[:])
            if dbg:
                MDC = cp.tile([128, NT], F32, tag="mdc", name="mdc")
                vec.tensor_copy(out=MDC[:], in_=MINDN[:])
                sync.dma_start(out=minddbg[:], in_=MDC[:])
                MWC = cp.tile([128, NT], F32, tag="mwc", name="mwc")
                vec.tensor_copy(out=MWC[:], in_=MINWN[:])
                sync.dma_start(out=minwdbg[:], in_=MWC[:])
                sync.dma_start(out=xtdbg[:], in_=XT[:])
                sync.dma_start(out=posdbg[:], in_=POSF[:])
                sync.dma_start(out=gvdbg[:], in_=GV[:])
            sync.dma_start(out=out[:], in_=PSB[:])
    nc.compile()
    return nc


# ------------------------------ host wrapper -------------------------------

def make_in_maps(boxes, agn_hm_pred, reg_pred):
    boxes = np.ascontiguousarray(np.asarray(boxes, np.float32))
    agn = np.ascontiguousarray(np.asarray(agn_hm_pred, np.float32))
    rp = np.ascontiguousarray(np.asarray(reg_pred, np.float32))
    agnfull = np.ascontiguousarray(agn.reshape(M_TOT, 1))
    in_maps = []
    for c in range(N_CORES):
        b, h = c // 2, c % 2
        idx = _SHARD_IDX[(b, h)]
        dyn = np.zeros((128, 5, NT), np.float32)
        a = np.zeros(NPAD, np.float32)
        a[:NV] = agn[idx]
        dyn[:, 0, :] = _pack(a)
        r = np.zeros((NPAD, 4), np.float32)
        r[:NV] = rp[idx]
        for k in range(4):
            dyn[:, 1 + k, :] = _pack(np.ascontiguousarray(r[:, k]))
        in_maps.append({
            "locst": _LOCSTAT[h],
            "locst16": _LOCSTAT16[h],
            "dyn": np.ascontiguousarray(dyn),
            "boxesT": np.ascontiguousarray(boxes[b].T),
            "boxesP": np.ascontiguousarray(np.tile(boxes[b], (2, 1))),
            "agnfull": agnfull,
            "corec": _corec(b, h),
            "miscc": _MISC,
        })
    return in_maps


_NC_CACHE = {}
LAST_RESULT = None


def _get_nc():
    if "nc" not in _NC_CACHE:
        _NC_CACHE["nc"] = build_nc(dbg=False)
    return _NC_CACHE["nc"]


def kernel(boxes, gt_classes=None, agn_hm_pred=None, reg_pred=None):
    global LAST_RESULT
    in_maps = make_in_maps(boxes, agn_hm_pred, reg_pred)
    nc = _get_nc()
    res = run_bass_kernel_spmd(nc, in_maps, core_ids=list(range(N_CORES)))
    LAST_RESULT = res
    parts = np.stack([np.asarray(r["out"], np.float64).reshape(8)
                      for r in res.results], 0).sum(0)
    rega, regc, poss, nega, npos = parts[0], parts[1], parts[2], parts[3], parts[4]
    npa = max(npos, 1.0)
    out = np.array([rega / max(regc, 1.0),
                    -0.125 * poss / npa,
                    -0.375 * nega / npa], np.float32)
    return out


# revision 26
# speedup vs baseline: 2.0880x; 1.1438x over previous
"""CenterNet loss (GT assignment + focal/giou losses) on 8 Trainium2 cores.

Sharding: core c handles image b = c//2 and half h = c%2 of EVERY FPN level
(so all 8 cores run an identical SPMD tile schedule). Each core produces
partial sums [1,8]: (giou_sum, reg_cnt, pos_sum, neg_sum, npos, ...); the
host sums the 8 partial vectors and computes the final 3-vector (the
"gather/unshard" step for scalar losses).

Device kernel (v2, fp16):
- pair loop in fp16: grid/cdis coords are integers (exact in fp16); the
  distance chain uses /32-scaled coordinates so squares stay in range.
- masks use additive -60000 penalties; d = -wdist2 + penalties, argmax.
- reg-target extraction: one-hot over boxes, transposed on the PE in
  [128,128] 2-tile blocks, then ONE fp16 2-split block-diagonal matmul
  per block extracts (x0,y0,-x1,-y1) of the argmin box per location.
"""

import numpy as np
import ml_dtypes

import concourse.bass as bass
import concourse.bacc as bacc
import concourse.tile as tile
from concourse import ap_utils, mybir
from concourse.bass_utils import run_bass_kernel_spmd


def _pool_on(eng, nc, out, in_, func):
    """Emit InstPool (innermost-dim reduction) on the given engine."""
    while len(in_.shape) < 5:
        in_ = in_.unsqueeze(1)
    return eng.add_instruction(mybir.InstPool(
        name=f"I-{nc.next_id()}", func=func,
        ins=[eng.lower_ap(in_, opt=False)], outs=[eng.lower_ap(out)]))

F32 = mybir.dt.float32
F16 = mybir.dt.float16
BF16 = mybir.dt.bfloat16
I32 = mybir.dt.int32
AF = mybir.ActivationFunctionType
OP = mybir.AluOpType
AX = mybir.AxisListType

# ---------------- problem constants (hardcoded from the nn.Module) ---------
B, NBOX = 4, 64
STRIDES = (8, 16, 32, 64, 128)
LEVEL_HW = ((128, 128), (64, 64), (32, 32), (16, 16), (8, 8))
SIZES = ((0.0, 80.0), (64.0, 160.0), (128.0, 320.0), (256.0, 640.0), (512.0, 1e7))
LOC = [h * w for h, w in LEVEL_HW]          # [16384, 4096, 1024, 256, 64]
M_IMG = sum(LOC)                            # 21824
M_TOT = B * M_IMG                           # 87296
BASE = [0, 65536, 81920, 86016, 87040]      # global level bases (level-major)
HALF = [m // 2 for m in LOC]                # per-core per-level loc counts
NT = 86                                     # 128-loc tiles per core
NV = sum(HALF)                              # 10912 valid locs per core
NPAD = NT * 128                             # 11008
MIN_RADIUS2 = 16.0
DELTA = (1 - 0.8) / (1 + 0.8)
K_R2 = float(np.float32(DELTA ** 2 * 2))    # radius2 = max(K_R2*area, 16)
SIG_LO = float(np.float32(1e-4))
SIG_HI = float(np.float32(1.0 - 1e-4))
EPS_AC = float(np.float32(1e-7))
IGNORE_HIGH_FP = 0.85
MAGIC = 8388608.0  # 2^23: u+MAGIC-MAGIC rounds u to nearest int (u < 2^22)
NEGK = -60000.0    # mask penalty (valid wdist2 is always < 2200)
CSC = 1.0 / 32.0   # distance-coordinate scale (keeps squares in fp16 range)
# supergroups: (tile0, n_tiles, level)
SG = [(0, 16, 0), (16, 16, 0), (32, 16, 0), (48, 16, 0),
      (64, 16, 1), (80, 4, 2), (84, 1, 3), (85, 1, 4)]

N_CORES = 8


def _pack(vec):
    """[NPAD] (loc j = t*128+p) -> [128, NT] with [p, t] layout."""
    return np.ascontiguousarray(vec.reshape(NT, 128).T)


def _grids_per_level():
    gs = []
    for (h, w), s in zip(LEVEL_HW, STRIDES):
        ys, xs = np.meshgrid(np.arange(h) * s, np.arange(w) * s, indexing="ij")
        g = np.stack([xs.reshape(-1), ys.reshape(-1)], 1).astype(np.float32) + s // 2
        gs.append(g)
    return gs


def _half_concat(per_level_fn, h):
    """Concat per-level arrays for half h, pad to NPAD."""
    parts = [per_level_fn(l, h) for l in range(5)]
    cat = np.concatenate(parts, 0)
    pad_shape = (NPAD - NV,) + cat.shape[1:]
    return np.concatenate([cat, np.zeros(pad_shape, cat.dtype)], 0)


_GRIDS = _grids_per_level()


def _build_locstat(h):
    """[128, 6, NT] f32: planes gx, gy, -gx, -gy, valid, inv_s."""
    g = _half_concat(lambda l, hh: _GRIDS[l][hh * HALF[l]:(hh + 1) * HALF[l]], h)
    gx, gy = g[:, 0], g[:, 1]
    valid = np.zeros(NPAD, np.float32)
    valid[:NV] = 1.0
    inv_s = _half_concat(
        lambda l, hh: np.full(HALF[l], 1.0 / STRIDES[l], np.float32), h)
    inv_s[NV:] = 1.0
    planes = [gx, gy, -gx, -gy, valid, inv_s]
    out = np.stack([_pack(p.astype(np.float32)) for p in planes], 1)
    return np.ascontiguousarray(out)  # [128, 6, NT]


def _build_locstat16(h):
    """[128, 4, NT] fp16: planes gx/32, gy/32, gx, gy."""
    g = _half_concat(lambda l, hh: _GRIDS[l][hh * HALF[l]:(hh + 1) * HALF[l]], h)
    gx, gy = g[:, 0], g[:, 1]
    planes = [gx * CSC, gy * CSC, gx, gy]
    out = np.stack([_pack(p.astype(np.float32)) for p in planes], 1)
    return np.ascontiguousarray(out.astype(np.float16))


_LOCSTAT = [_build_locstat(0), _build_locstat(1)]
_LOCSTAT16 = [_build_locstat16(0), _build_locstat16(1)]

# iota constants [128, 2, 64] bf16: plane0 = 0..63, plane1 = 64 - iota
_MISC = np.ascontiguousarray(np.stack([
    np.broadcast_to(np.arange(64, dtype=np.float32), (128, 64)),
    np.broadcast_to(64.0 - np.arange(64, dtype=np.float32), (128, 64)),
], 1)).astype(ml_dtypes.bfloat16)


def _shard_idx(b, h):
    """Global level-major indices of core (b, h)'s NV locations."""
    parts = [BASE[l] + b * LOC[l] + h * HALF[l] + np.arange(HALF[l])
             for l in range(5)]
    return np.concatenate(parts, 0)


_SHARD_IDX = {(b, h): _shard_idx(b, h) for b in range(B) for h in range(2)}


def _corec(b, h):
    """[NBOX, 28] f32 per-core consts for the pos part:
    cols 0-4 global level base (+image), 5 even-core gate, 6-10 1/s,
    11-15 level width, 16-20 lo, 21-25 hi."""
    c = np.zeros((NBOX, 28), np.float32)
    for l in range(5):
        c[:, l] = BASE[l] + b * LOC[l]
        c[:, 6 + l] = 1.0 / STRIDES[l]
        c[:, 11 + l] = LEVEL_HW[l][1]
        c[:, 16 + l] = SIZES[l][0]
        c[:, 21 + l] = SIZES[l][1]
    c[:, 5] = 1.0 if h == 0 else 0.0
    return c


# ------------------------------ device program -----------------------------

def build_nc(dbg=False):
    nc = bacc.Bacc(trn_type="TRN2", num_devices=N_CORES)
    locst = nc.dram_tensor("locst", [128, 6, NT], F32, kind="ExternalInput")
    locst16 = nc.dram_tensor("locst16", [128, 4, NT], F16, kind="ExternalInput")
    dyn = nc.dram_tensor("dyn", [128, 5, NT], F32, kind="ExternalInput")
    boxesT = nc.dram_tensor("boxesT", [4, NBOX], F32, kind="ExternalInput")
    boxesP = nc.dram_tensor("boxesP", [2 * NBOX, 4], F32, kind="ExternalInput")
    agnfull = nc.dram_tensor("agnfull", [M_TOT, 1], F32, kind="ExternalInput")
    corec = nc.dram_tensor("corec", [NBOX, 28], F32, kind="ExternalInput")
    out = nc.dram_tensor("out", [1, 8], F32, kind="ExternalOutput")
    if dbg:
        minddbg = nc.dram_tensor("minddbg", [128, NT], F32, kind="ExternalOutput")
        minwdbg = nc.dram_tensor("minwdbg", [128, NT], F32, kind="ExternalOutput")
        xtdbg = nc.dram_tensor("xtdbg", [128, 4, NT], F32, kind="ExternalOutput")
        posdbg = nc.dram_tensor("posdbg", [NBOX, 5], F32, kind="ExternalOutput")
        gvdbg = nc.dram_tensor("gvdbg", [NBOX, 5], F32, kind="ExternalOutput")

    vec, act, gps, sync = nc.vector, nc.scalar, nc.gpsimd, nc.sync

    with tile.TileContext(nc) as tc:
        with tc.tile_pool(name="const", bufs=1) as cp, \
             tc.tile_pool(name="work", bufs=3) as wp, \
             tc.tile_pool(name="ppool", bufs=2, space="PSUM") as pp:

            # ---------------- loads ----------------
            SL = cp.tile([128, 6, NT], F32)
            sync.dma_start(out=SL[:], in_=locst[:])
            SL16 = cp.tile([128, 4, NT], F16)
            sync.dma_start(out=SL16[:], in_=locst16[:])
            DY = cp.tile([128, 5, NT], F32)
            sync.dma_start(out=DY[:], in_=dyn[:])
            BBh = boxesT[:]
            BB = cp.tile([128, 4, NBOX], F32)
            bt_bc = bass.AP(tensor=BBh.tensor, offset=BBh.offset,
                            ap=[[0, 128], [NBOX, 4], [1, NBOX]])
            sync.dma_start(out=BB[:], in_=bt_bc)
            BP = cp.tile([2 * NBOX, 4], F32)
            sync.dma_start(out=BP[:], in_=boxesP[:])
            CO = cp.tile([NBOX, 28], F32)
            sync.dma_start(out=CO[:], in_=corec[:])

            # ---------------- per-box precompute ([128, 64] broadcast) -----
            from concourse.masks import make_identity
            IDT = cp.tile([128, 128], BF16)
            make_identity(nc, IDT[:])
            x0, y0 = BB[:, 0, :], BB[:, 1, :]
            x1, y1 = BB[:, 2, :], BB[:, 3, :]
            CXY = cp.tile([128, 2, NBOX], F32)
            CX, CY = CXY[:, 0, :], CXY[:, 1, :]
            vec.tensor_tensor(out=CXY[:], in0=BB[:, 0:2, :], in1=BB[:, 2:4, :],
                              op=OP.add)
            vec.tensor_scalar(out=CXY[:], in0=CXY[:], scalar1=0.5, scalar2=None,
                              op0=OP.mult)
            # W2C = (w/2, h/2) per box;  IW2 = 32/w2 (fp16, scaled units)
            W2C = cp.tile([128, 2, NBOX], F32)
            vec.tensor_tensor(out=W2C[:], in0=BB[:, 2:4, :], in1=BB[:, 0:2, :],
                              op=OP.subtract)
            vec.tensor_scalar(out=W2C[:], in0=W2C[:], scalar1=0.5, scalar2=None,
                              op0=OP.mult)
            W2SF = cp.tile([128, 2, NBOX], F32)
            vec.tensor_scalar(out=W2SF[:], in0=W2C[:], scalar1=CSC,
                              scalar2=None, op0=OP.mult)
            W2S2 = cp.tile([128, 2, NBOX], F16)
            vec.tensor_tensor(out=W2S2[:], in0=W2SF[:], in1=W2SF[:],
                              op=OP.mult)
            S1 = cp.tile([128, NBOX], F32)
            S2 = cp.tile([128, NBOX], F32)
            S3 = cp.tile([128, NBOX], F32)
            # radius2 = max(K_R2 * area, 16);  IR2N = -1024/radius2 (fp16)
            R2 = cp.tile([128, NBOX], F32)
            vec.tensor_tensor(out=S1[:], in0=x1, in1=x0, op=OP.subtract)  # w
            vec.tensor_tensor(out=S2[:], in0=y1, in1=y0, op=OP.subtract)  # h
            vec.tensor_tensor(out=R2[:], in0=S1[:], in1=S2[:], op=OP.mult)
            vec.tensor_scalar(out=R2[:], in0=R2[:], scalar1=K_R2,
                              scalar2=MIN_RADIUS2, op0=OP.mult, op1=OP.max)
            IR2F = cp.tile([128, NBOX], F32)
            vec.reciprocal(out=IR2F[:], in_=R2[:])
            IR2N = cp.tile([128, NBOX], F16)
            vec.tensor_scalar(out=IR2N[:], in0=IR2F[:], scalar1=-1024.0,
                              scalar2=None, op0=OP.mult)
            # crit = sqrt(w^2 + h^2) / 2  (per box)
            CRIT = cp.tile([128, NBOX], F32)
            vec.tensor_tensor(out=S1[:], in0=S1[:], in1=S1[:], op=OP.mult)
            vec.tensor_tensor(out=S2[:], in0=S2[:], in1=S2[:], op=OP.mult)
            vec.tensor_tensor(out=S1[:], in0=S1[:], in1=S2[:], op=OP.add)
            act.activation(out=CRIT[:], in_=S1[:], func=AF.Sqrt, scale=0.25)
            # PBS [128, 4] = (x0, y0, -x1, -y1), both 64-box halves identical
            PBS = cp.tile([2 * NBOX, 4], F32)
            vec.tensor_copy(out=PBS[:, 0:2], in_=BP[:, 0:2])
            vec.tensor_scalar(out=PBS[:, 2:4], in0=BP[:, 2:4], scalar1=-1.0,
                              scalar2=None, op0=OP.mult)
            # bf16 2-split of PBS into block-diagonal rhs PBSD [128, 16]:
            # rows 0:64 -> cols 0:4 (hi), 8:12 (lo); rows 64:128 -> 4:8, 12:16
            PBSH = cp.tile([2 * NBOX, 4], BF16)
            vec.tensor_copy(out=PBSH[:], in_=PBS[:])
            PBSR = cp.tile([2 * NBOX, 4], F32)
            vec.tensor_copy(out=PBSR[:], in_=PBSH[:])
            PBSL = cp.tile([2 * NBOX, 4], BF16)
            vec.tensor_tensor(out=PBSL[:], in0=PBS[:], in1=PBSR[:],
                              op=OP.subtract)
            PBSD = cp.tile([128, 16], BF16)
            vec.memset(PBSD[:], 0.0)
            vec.tensor_copy(out=PBSD[0:NBOX, 0:4], in_=PBSH[0:NBOX, :])
            vec.tensor_copy(out=PBSD[NBOX:128, 4:8], in_=PBSH[NBOX:128, :])
            vec.tensor_copy(out=PBSD[0:NBOX, 8:12], in_=PBSL[0:NBOX, :])
            vec.tensor_copy(out=PBSD[NBOX:128, 12:16], in_=PBSL[NBOX:128, :])
            # per-level consts: BCNN16[l] = carednot penalty {0,-60000(x2)};
            # CB4_16[l] = (cx/32, cy/32, cdisx, cdisy) fp16
            BCNN, CB4 = [], []
            for l in range(5):
                s = float(STRIDES[l])
                lo, hi = SIZES[l]
                vec.tensor_scalar(out=S1[:], in0=CRIT[:], scalar1=float(lo),
                                  scalar2=NEGK, op0=OP.is_lt, op1=OP.mult)
                vec.tensor_scalar(out=S2[:], in0=CRIT[:], scalar1=float(hi),
                                  scalar2=NEGK, op0=OP.is_gt, op1=OP.mult)
                cn = cp.tile([128, NBOX], F16, tag=f"bcnn{l}", name=f"bcnn{l}")
                vec.tensor_tensor(out=cn[:], in0=S1[:], in1=S2[:], op=OP.add)
                BCNN.append(cn)
                t = cp.tile([128, 4, NBOX], F16, tag=f"cb4{l}", name=f"cb4{l}")
                vec.tensor_scalar(out=t[:, 0:2, :], in0=CXY[:], scalar1=CSC,
                                  scalar2=None, op0=OP.mult)
                for ci, src_ in enumerate((CX, CY)):
                    vec.tensor_scalar(out=S1[:], in0=src_, scalar1=1.0 / s,
                                      scalar2=None, op0=OP.mult)  # u = c/s
                    vec.tensor_scalar(out=S2[:], in0=S1[:], scalar1=MAGIC,
                                      scalar2=None, op0=OP.add)
                    vec.tensor_scalar(out=S2[:], in0=S2[:], scalar1=-MAGIC,
                                      scalar2=None, op0=OP.add)
                    vec.tensor_tensor(out=S3[:], in0=S2[:], in1=S1[:], op=OP.is_gt)
                    vec.tensor_tensor(out=S2[:], in0=S2[:], in1=S3[:],
                                      op=OP.subtract)  # floor(c/s)
                    vec.tensor_scalar(out=t[:, 2 + ci, :], in0=S2[:], scalar1=s,
                                      scalar2=s / 2.0, op0=OP.mult, op1=OP.add)
                CB4.append(t)

            # ---------------- pos part (boxes on partitions, [64, 5]) -------
            cx = cp.tile([NBOX, 1], F32, tag="pcx", name="pcx")
            cy = cp.tile([NBOX, 1], F32, tag="pcy", name="pcy")
            vec.tensor_tensor(out=cx[:], in0=BP[0:NBOX, 0:1], in1=BP[0:NBOX, 2:3], op=OP.add)
            vec.tensor_scalar(out=cx[:], in0=cx[:], scalar1=0.5, scalar2=None,
                              op0=OP.mult)
            vec.tensor_tensor(out=cy[:], in0=BP[0:NBOX, 1:2], in1=BP[0:NBOX, 3:4], op=OP.add)
            vec.tensor_scalar(out=cy[:], in0=cy[:], scalar1=0.5, scalar2=None,
                              op0=OP.mult)
            pw = cp.tile([NBOX, 1], F32, tag="ppw", name="ppw")
            ph = cp.tile([NBOX, 1], F32, tag="pph", name="pph")
            vec.tensor_tensor(out=pw[:], in0=BP[0:NBOX, 2:3], in1=BP[0:NBOX, 0:1],
                              op=OP.subtract)
            vec.tensor_tensor(out=ph[:], in0=BP[0:NBOX, 3:4], in1=BP[0:NBOX, 1:2],
                              op=OP.subtract)
            vec.tensor_tensor(out=pw[:], in0=pw[:], in1=pw[:], op=OP.mult)
            vec.tensor_tensor(out=ph[:], in0=ph[:], in1=ph[:], op=OP.mult)
            vec.tensor_tensor(out=pw[:], in0=pw[:], in1=ph[:], op=OP.add)
            pcrit = cp.tile([NBOX, 1], F32, tag="pcrit", name="pcrit")
            act.activation(out=pcrit[:], in_=pw[:], func=AF.Sqrt, scale=0.25)

            def p5(tag):
                return cp.tile([NBOX, 5], F32, tag=tag, name=tag)

            PA5, PB5, PC5, PD5, PE5 = (p5("pa5"), p5("pb5"), p5("pc5"),
                                       p5("pd5"), p5("pe5"))
            ISL, WLV = CO[:, 6:11], CO[:, 11:16]
            LOV, HIV = CO[:, 16:21], CO[:, 21:26]
            # ci_x = floor(cx/s) for all 5 levels at once
            vec.tensor_tensor(out=PD5[:], in0=cx[:].broadcast_to((NBOX, 5)),
                              in1=ISL, op=OP.mult)
            vec.tensor_scalar(out=PA5[:], in0=PD5[:], scalar1=MAGIC,
                              scalar2=-MAGIC, op0=OP.add, op1=OP.add)
            vec.tensor_tensor(out=PE5[:], in0=PA5[:], in1=PD5[:], op=OP.is_gt)
            vec.tensor_tensor(out=PA5[:], in0=PA5[:], in1=PE5[:], op=OP.subtract)
            # ci_y
            vec.tensor_tensor(out=PD5[:], in0=cy[:].broadcast_to((NBOX, 5)),
                              in1=ISL, op=OP.mult)
            vec.tensor_scalar(out=PB5[:], in0=PD5[:], scalar1=MAGIC,
                              scalar2=-MAGIC, op0=OP.add, op1=OP.add)
            vec.tensor_tensor(out=PE5[:], in0=PB5[:], in1=PD5[:], op=OP.is_gt)
            vec.tensor_tensor(out=PB5[:], in0=PB5[:], in1=PE5[:], op=OP.subtract)
            # pos = base + ci_y*W + ci_x, clamped
            vec.tensor_tensor(out=PC5[:], in0=PB5[:], in1=WLV, op=OP.mult)
            vec.tensor_tensor(out=PC5[:], in0=PC5[:], in1=PA5[:], op=OP.add)
            vec.tensor_tensor(out=PC5[:], in0=PC5[:], in1=CO[:, 0:5], op=OP.add)
            POSF = p5("posf")
            vec.tensor_scalar(out=POSF[:], in0=PC5[:], scalar1=0.0,
                              scalar2=float(M_TOT - 1), op0=OP.max, op1=OP.min)
            # pos mask
            PM = p5("pm")
            vec.tensor_tensor(out=PA5[:], in0=pcrit[:].broadcast_to((NBOX, 5)),
                              in1=LOV, op=OP.is_ge)
            vec.tensor_tensor(out=PB5[:], in0=pcrit[:].broadcast_to((NBOX, 5)),
                              in1=HIV, op=OP.is_le)
            vec.tensor_tensor(out=PM[:], in0=PA5[:], in1=PB5[:], op=OP.mult)
            POSI = cp.tile([NBOX, 5], I32, tag="posi", name="posi")
            vec.tensor_copy(out=POSI[:], in_=POSF[:])
            GV = cp.tile([NBOX, 5], F32, tag="gv", name="gv")
            for l in range(5):
                gps.indirect_dma_start(
                    out=GV[:, l:l + 1], out_offset=None, in_=agnfull[:],
                    in_offset=bass.IndirectOffsetOnAxis(ap=POSI[:, l:l + 1],
                                                        axis=0))
            PPRED = cp.tile([NBOX, 5], F32, tag="ppred", name="ppred")
            act.activation(out=PPRED[:], in_=GV[:], func=AF.Sigmoid)
            vec.tensor_scalar(out=PPRED[:], in0=PPRED[:], scalar1=SIG_LO,
                              scalar2=SIG_HI, op0=OP.max, op1=OP.min)
            QQ = cp.tile([NBOX, 5], F32, tag="qq", name="qq")
            vec.tensor_scalar(out=QQ[:], in0=PPRED[:], scalar1=-1.0, scalar2=1.0,
                              op0=OP.mult, op1=OP.add)
            vec.tensor_tensor(out=QQ[:], in0=QQ[:], in1=QQ[:], op=OP.mult)
            LGP = cp.tile([NBOX, 5], F32, tag="lgp", name="lgp")
            act.activation(out=LGP[:], in_=PPRED[:], func=AF.Ln)
            vec.tensor_tensor(out=LGP[:], in0=LGP[:], in1=QQ[:], op=OP.mult)
            vec.tensor_tensor(out=LGP[:], in0=LGP[:], in1=PM[:], op=OP.mult)
            # gate odd cores to zero (pos part owned by even core of each image)
            vec.tensor_scalar(out=LGP[:], in0=LGP[:], scalar1=CO[:, 5:6],
                              scalar2=None, op0=OP.mult)
            vec.tensor_scalar(out=PM[:], in0=PM[:], scalar1=CO[:, 5:6],
                              scalar2=None, op0=OP.mult)
            POSS = cp.tile([NBOX, 1], F32, tag="poss", name="poss")
            vec.tensor_reduce(out=POSS[:], in_=LGP[:], axis=AX.X, op=OP.add)
            NPOS = cp.tile([NBOX, 1], F32, tag="npos", name="npos")
            vec.tensor_reduce(out=NPOS[:], in_=PM[:], axis=AX.X, op=OP.add)

            # ---------------- main pair loop (fp16) -------------------------
            # negated convention: MINWN = -min(wdist2) = max(-wdist2),
            # MINDN = max(DN) where DN = -wdist2 + penalties,
            # NIXP = 64 - argmin(d).
            MINWN = cp.tile([128, NT], F16)
            MINDN = cp.tile([128, NT], F16)
            OH2 = cp.tile([128, 2, 64], BF16)     # one-hot for tiles 84+85
            XT = cp.tile([128, 4, NT], F32)       # selected (x0, y0, -x1, -y1)

            def rt_blocks(oh_tile, lb0, gt0, nblk):
                """Extract reg targets for `nblk` 2-tile blocks.

                oh_tile: one-hot tile [128, *, 64]; local tile lb0 onward.
                gt0: global tile index of oh_tile[:, lb0]."""
                for g4 in range(0, nblk, 4):
                    blks = range(g4, min(g4 + 4, nblk))
                    nb = len(blks)
                    RTP4 = pp.tile([128, 64], F32, tag="rtp4", name="rtp4")
                    for j, blk in enumerate(blks):
                        OHT = pp.tile([128, 128], BF16, tag="oht", name="oht")
                        base = oh_tile[:, lb0 + 2 * blk, :]
                        oh_blk = bass.AP(
                            tensor=base.tensor, offset=base.offset,
                            ap=[list(base.ap[0]), [1, 128]])
                        nc.tensor.transpose(OHT[:], oh_blk, IDT[:])
                        OHS = wp.tile([128, 128], BF16, tag="ohs", name="ohs")
                        act.copy(out=OHS[:], in_=OHT[:])
                        nc.tensor.matmul(out=RTP4[:, 16 * j:16 * (j + 1)],
                                         lhsT=OHS[:], rhs=PBSD[:],
                                         start=True, stop=True)
                    # combine hi+lo splits: XT[:, c, gt0+2blk+ab] = hi + lo
                    RTS = wp.tile([128, 64], F32, tag="rts", name="rts")
                    act.copy(out=RTS[:], in_=RTP4[:])
                    t0_ = gt0 + 2 * g4
                    xt_base = XT[:, :, t0_:t0_ + 2 * nb]
                    xt_out = bass.AP(
                        tensor=xt_base.tensor, offset=xt_base.offset,
                        ap=[list(xt_base.ap[0]), [NT, 4], [2, nb], [1, 2]])
                    hi_base = RTS[:]
                    hi_in = bass.AP(
                        tensor=hi_base.tensor, offset=hi_base.offset,
                        ap=[list(hi_base.ap[0]), [1, 4], [16, nb], [4, 2]])
                    lo_base = RTS[:, 8:]
                    lo_in = bass.AP(
                        tensor=lo_base.tensor, offset=lo_base.offset,
                        ap=[list(lo_base.ap[0]), [1, 4], [16, nb], [4, 2]])
                    gps.tensor_tensor(out=xt_out, in0=hi_in, in1=lo_in,
                                      op=OP.add)

            for (t0, G, l) in SG:
                s = float(STRIDES[l])
                sl = slice(t0, t0 + G)

                def bb1(t2d, n=1):  # [128,n,64] const -> [128,n,G,64]
                    if n == 1:
                        return t2d.unsqueeze(1).broadcast_to((128, G, 64))
                    return t2d.unsqueeze(2).broadcast_to((128, n, G, 64))

                # DF = (dx', dy', dxc, dyc): scaled center diffs + cdis diffs
                DF = wp.tile([128, 4, G, 64], F16, tag="df", name="df")
                vec.tensor_tensor(
                    out=DF[:],
                    in0=SL16[:, 0:4, sl].unsqueeze(3).broadcast_to((128, 4, G, 64)),
                    in1=bb1(CB4[l][:], 4), op=OP.subtract)
                # squares of all 4 diff planes (act engine)
                SQ = wp.tile([128, 4, G, 64], F16, tag="sq", name="sq")
                act.square(out=SQ[:], in_=DF[:])
                # in-box test: dx'^2 >= (w2/32)^2 (either axis) -> penalty
                C2 = wp.tile([128, 2, G, 64], F16, tag="c2", name="c2")
                vec.tensor_tensor(out=C2[:], in0=SQ[:, 0:2],
                                  in1=bb1(W2S2[:], 2), op=OP.is_ge)
                PIN = wp.tile([128, G, 64], F16, tag="pin", name="pin")
                vec.tensor_tensor(out=PIN[:], in0=C2[:, 0], in1=C2[:, 1],
                                  op=OP.max)
                # 3x3 test: max(dxc^2, dyc^2) > s^2 -> penalty; == 0 -> peak
                M = wp.tile([128, G, 64], F16, tag="m", name="m")
                vec.tensor_tensor(out=M[:], in0=SQ[:, 2], in1=SQ[:, 3],
                                  op=OP.max)
                P3 = wp.tile([128, G, 64], F16, tag="p3", name="p3")
                vec.tensor_scalar(out=P3[:], in0=M[:], scalar1=s * s,
                                  scalar2=NEGK, op0=OP.is_gt, op1=OP.mult)
                PINP = wp.tile([128, G, 64], F16, tag="pinp", name="pinp")
                vec.tensor_scalar(out=PINP[:], in0=PIN[:], scalar1=NEGK,
                                  scalar2=None, op0=OP.mult)
                PB = wp.tile([128, G, 64], F16, tag="pb", name="pb")
                vec.tensor_tensor(out=PB[:], in0=P3[:], in1=bb1(BCNN[l][:]),
                                  op=OP.add)
                # dist2 (scaled) with peak zeroing, then * (-1024/r2)
                D2 = wp.tile([128, G, 64], F16, tag="d2", name="d2")
                vec.tensor_tensor(out=D2[:], in0=SQ[:, 0], in1=SQ[:, 1],
                                  op=OP.add)
                NPK = wp.tile([128, G, 64], F16, tag="npk", name="npk")
                vec.tensor_scalar(out=NPK[:], in0=M[:], scalar1=0.0,
                                  scalar2=None, op0=OP.not_equal)
                DZ = wp.tile([128, G, 64], F16, tag="dz", name="dz")
                vec.tensor_tensor(out=DZ[:], in0=D2[:], in1=NPK[:], op=OP.mult)
                WDN = wp.tile([128, G, 64], F16, tag="wdn", name="wdn")
                vec.tensor_tensor(out=WDN[:], in0=DZ[:], in1=bb1(IR2N[:]),
                                  op=OP.mult)  # -wdist2
                vec.tensor_reduce(out=MINWN[:, sl], in_=WDN[:], axis=AX.X,
                                  op=OP.max)
                T9 = wp.tile([128, G, 64], F16, tag="t9", name="t9")
                vec.tensor_tensor(out=T9[:], in0=WDN[:], in1=PINP[:], op=OP.add)
                DN = wp.tile([128, G, 64], F16, tag="dn", name="dn")
                vec.tensor_tensor(out=DN[:], in0=T9[:], in1=PB[:], op=OP.add)
                vec.tensor_reduce(out=MINDN[:, sl], in_=DN[:], axis=AX.X,
                                  op=OP.max)
                # one-hot of the argmin box (exact fp16 ties are rare enough
                # that multi-hot rows stay inside the error tolerance)
                if G > 1:
                    EQ = wp.tile([128, G, 64], BF16, tag="eq", name="eq")
                    eq_dst = EQ[:]
                else:
                    eq_dst = OH2[:, t0 - 84:t0 - 83, :]
                vec.tensor_tensor(out=eq_dst, in0=DN[:],
                                  in1=MINDN[:, sl].unsqueeze(2).broadcast_to(
                                      (128, G, 64)), op=OP.is_equal)
                if G > 1:
                    rt_blocks(EQ, 0, t0, G // 2)

            # tiles 84 (L3) + 85 (L4) share one 2-tile block
            rt_blocks(OH2, 0, 84, 1)

            # ---------------- epilogue: per-location [128, NT] --------------
            AGN = DY[:, 0, :]
            VAL = SL[:, 4, :]
            ISV = SL[:, 5, :]

            def lt(tag):
                return wp.tile([128, NT], F32, tag=tag, name=tag)

            HM = lt("hm")
            act.activation(out=HM[:], in_=MINWN[:], func=AF.Exp, scale=1.0)
            vec.scalar_tensor_tensor(out=HM[:], in0=HM[:], scalar=SIG_LO,
                                     in1=HM[:], op0=OP.is_ge, op1=OP.mult)
            NW = lt("nw")
            vec.tensor_scalar(out=NW[:], in0=HM[:], scalar1=-1.0, scalar2=1.0,
                              op0=OP.mult, op1=OP.add)
            vec.tensor_tensor(out=NW[:], in0=NW[:], in1=NW[:], op=OP.mult)
            vec.tensor_tensor(out=NW[:], in0=NW[:], in1=NW[:], op=OP.mult)
            PC = lt("pc")
            act.activation(out=PC[:], in_=AGN, func=AF.Sigmoid)
            vec.tensor_scalar(out=PC[:], in0=PC[:], scalar1=SIG_LO,
                              scalar2=SIG_HI, op0=OP.max, op1=OP.min)
            Q = lt("q")
            vec.tensor_scalar(out=Q[:], in0=PC[:], scalar1=-1.0, scalar2=1.0,
                              op0=OP.mult, op1=OP.add)
            act.activation(out=Q[:], in_=Q[:], func=AF.Ln)  # log(1-pred)
            P2 = lt("p2")
            vec.tensor_tensor(out=P2[:], in0=PC[:], in1=PC[:], op=OP.mult)
            T1 = lt("t1")
            vec.tensor_tensor(out=T1[:], in0=Q[:], in1=P2[:], op=OP.mult)
            vec.tensor_tensor(out=T1[:], in0=T1[:], in1=NW[:], op=OP.mult)
            GT = lt("gt")
            vec.tensor_scalar(out=GT[:], in0=PC[:], scalar1=IGNORE_HIGH_FP,
                              scalar2=None, op0=OP.is_lt)
            vec.tensor_tensor(out=T1[:], in0=T1[:], in1=GT[:], op=OP.mult)
            vec.tensor_tensor(out=T1[:], in0=T1[:], in1=VAL, op=OP.mult)
            NEGA = cp.tile([128, 1], F32)
            vec.tensor_reduce(out=NEGA[:], in_=T1[:], axis=AX.X, op=OP.add)
            # validity + rt
            VM = lt("vm")
            vec.tensor_scalar(out=VM[:], in0=MINDN[:], scalar1=-50000.0,
                              scalar2=None, op0=OP.is_gt)
            vec.tensor_tensor(out=VM[:], in0=VM[:], in1=VAL, op=OP.mult)
            REGC = cp.tile([128, 1], F32)
            vec.tensor_reduce(out=REGC[:], in_=VM[:], axis=AX.X, op=OP.add)
            RT = wp.tile([128, 4, NT], F32, tag="rt", name="rt")
            vec.scalar_tensor_tensor(out=RT[:, 0:2, :], in0=XT[:, 0:2, :],
                                     scalar=-1.0, in1=SL[:, 0:2, :],
                                     op0=OP.mult, op1=OP.add)
            vec.scalar_tensor_tensor(out=RT[:, 2:4, :], in0=XT[:, 2:4, :],
                                     scalar=-1.0, in1=SL[:, 2:4, :],
                                     op0=OP.mult, op1=OP.add)
            # RT = signed_grid - XT = (l, t, r, b) of argmin box; / stride
            vec.tensor_tensor(out=RT[:], in0=RT[:],
                              in1=ISV.unsqueeze(1).broadcast_to((128, 4, NT)),
                              op=OP.mult)
            # rtf = rt*vm + (1-vm)   (exact select; vm in {0,1})
            RTF = wp.tile([128, 4, NT], F32, tag="rtf", name="rtf")
            vec.tensor_tensor(out=RTF[:], in0=RT[:],
                              in1=VM[:].unsqueeze(1).broadcast_to((128, 4, NT)),
                              op=OP.mult)
            VMN = lt("vmn")
            vec.tensor_scalar(out=VMN[:], in0=VM[:], scalar1=-1.0, scalar2=1.0,
                              op0=OP.mult, op1=OP.add)
            vec.tensor_tensor(out=RTF[:], in0=RTF[:],
                              in1=VMN[:].unsqueeze(1).broadcast_to((128, 4, NT)),
                              op=OP.add)
            # giou(pred, rtf)
            pl, pt = DY[:, 1, :], DY[:, 2, :]
            pr, pb = DY[:, 3, :], DY[:, 4, :]
            tl, tt_ = RTF[:, 0, :], RTF[:, 1, :]
            tr, tb = RTF[:, 2, :], RTF[:, 3, :]
            TA, PA, WI, GW, HI, GH = (lt("ta"), lt("pa"), lt("wi"), lt("gw"),
                                      lt("hi"), lt("gh"))
            SA, SB = lt("sa"), lt("sb")
            vec.tensor_tensor(out=SA[:], in0=tl, in1=tr, op=OP.add)
            vec.tensor_tensor(out=SB[:], in0=tt_, in1=tb, op=OP.add)
            vec.tensor_tensor(out=TA[:], in0=SA[:], in1=SB[:], op=OP.mult)
            vec.tensor_tensor(out=SA[:], in0=pl, in1=pr, op=OP.add)
            vec.tensor_tensor(out=SB[:], in0=pt, in1=pb, op=OP.add)
            vec.tensor_tensor(out=PA[:], in0=SA[:], in1=SB[:], op=OP.mult)
            vec.tensor_tensor(out=SA[:], in0=pl, in1=tl, op=OP.min)
            vec.tensor_tensor(out=SB[:], in0=pr, in1=tr, op=OP.min)
            vec.tensor_tensor(out=WI[:], in0=SA[:], in1=SB[:], op=OP.add)
            vec.tensor_tensor(out=SA[:], in0=pl, in1=tl, op=OP.max)
            vec.tensor_tensor(out=SB[:], in0=pr, in1=tr, op=OP.max)
            vec.tensor_tensor(out=GW[:], in0=SA[:], in1=SB[:], op=OP.add)
            vec.tensor_tensor(out=SA[:], in0=pb, in1=tb, op=OP.min)
            vec.tensor_tensor(out=SB[:], in0=pt, in1=tt_, op=OP.min)
            vec.tensor_tensor(out=HI[:], in0=SA[:], in1=SB[:], op=OP.add)
            vec.tensor_tensor(out=SA[:], in0=pb, in1=tb, op=OP.max)
            vec.tensor_tensor(out=SB[:], in0=pt, in1=tt_, op=OP.max)
            vec.tensor_tensor(out=GH[:], in0=SA[:], in1=SB[:], op=OP.add)
            AC = lt("ac")
            vec.tensor_tensor(out=AC[:], in0=GW[:], in1=GH[:], op=OP.mult)
            vec.tensor_scalar(out=AC[:], in0=AC[:], scalar1=EPS_AC,
                              scalar2=None, op0=OP.add)
            INTER = lt("inter")
            vec.tensor_tensor(out=INTER[:], in0=WI[:], in1=HI[:], op=OP.mult)
            UN = lt("un")
            vec.tensor_tensor(out=UN[:], in0=TA[:], in1=PA[:], op=OP.add)
            vec.tensor_tensor(out=UN[:], in0=UN[:], in1=INTER[:], op=OP.subtract)
            vec.tensor_scalar(out=SA[:], in0=INTER[:], scalar1=1.0,
                              scalar2=None, op0=OP.add)
            vec.tensor_scalar(out=SB[:], in0=UN[:], scalar1=1.0,
                              scalar2=None, op0=OP.add)
            IOU = lt("iou")
            vec.reciprocal(out=SB[:], in_=SB[:])
            vec.tensor_tensor(out=IOU[:], in0=SA[:], in1=SB[:], op=OP.mult)
            vec.tensor_tensor(out=SA[:], in0=AC[:], in1=UN[:], op=OP.subtract)
            vec.reciprocal(out=SB[:], in_=AC[:])
            vec.tensor_tensor(out=SB[:], in0=SA[:], in1=SB[:], op=OP.mult)
            vec.tensor_tensor(out=IOU[:], in0=IOU[:], in1=SB[:], op=OP.subtract)
            vec.tensor_scalar(out=IOU[:], in0=IOU[:], scalar1=-1.0, scalar2=1.0,
                              op0=OP.mult, op1=OP.add)  # 1 - giou
            vec.tensor_tensor(out=IOU[:], in0=IOU[:], in1=VM[:], op=OP.mult)
            REGA = cp.tile([128, 1], F32)
            vec.tensor_reduce(out=REGA[:], in_=IOU[:], axis=AX.X, op=OP.add)

            # ---------------- partial reduction + output --------------------
            PART = cp.tile([128, 8], F32)
            vec.memset(PART[:], 0.0)
            vec.tensor_copy(out=PART[:, 0:1], in_=REGA[:])
            vec.tensor_copy(out=PART[:, 1:2], in_=REGC[:])
            vec.tensor_copy(out=PART[:, 3:4], in_=NEGA[:])
            vec.tensor_copy(out=PART[0:NBOX, 2:3], in_=POSS[:])
            vec.tensor_copy(out=PART[0:NBOX, 4:5], in_=NPOS[:])
            ONES = cp.tile([128, 1], F32)
            vec.memset(ONES[:], 1.0)
            PS = pp.tile([1, 8], F32, bufs=1)
            nc.tensor.matmul(out=PS[:], lhsT=ONES[:], rhs=PART[:],
                             start=True, stop=True)
            PSB = cp.tile([1, 8], F32)
            vec.tensor_copy(out=PSB[:], in_=PS[:])
            if dbg:
                MDC = cp.tile([128, NT], F32, tag="mdc", name="mdc")
                vec.tensor_copy(out=MDC[:], in_=MINDN[:])
                sync.dma_start(out=minddbg[:], in_=MDC[:])
                MWC = cp.tile([128, NT], F32, tag="mwc", name="mwc")
                vec.tensor_copy(out=MWC[:], in_=MINWN[:])
                sync.dma_start(out=minwdbg[:], in_=MWC[:])
                sync.dma_start(out=xtdbg[:], in_=XT[:])
                sync.dma_start(out=posdbg[:], in_=POSF[:])
                sync.dma_start(out=gvdbg[:], in_=GV[:])
            sync.dma_start(out=out[:], in_=PSB[:])
    nc.compile()
    return nc


# ------------------------------ host wrapper -------------------------------

def make_in_maps(boxes, agn_hm_pred, reg_pred):
    boxes = np.ascontiguousarray(np.asarray(boxes, np.float32))
    agn = np.ascontiguousarray(np.asarray(agn_hm_pred, np.float32))
    rp = np.ascontiguousarray(np.asarray(reg_pred, np.float32))
    agnfull = np.ascontiguousarray(agn.reshape(M_TOT, 1))
    in_maps = []
    for c in range(N_CORES):
        b, h = c // 2, c % 2
        idx = _SHARD_IDX[(b, h)]
        dyn = np.zeros((128, 5, NT), np.float32)
        a = np.zeros(NPAD, np.float32)
        a[:NV] = agn[idx]
        dyn[:, 0, :] = _pack(a)
        r = np.zeros((NPAD, 4), np.float32)
        r[:NV] = rp[idx]
        for k in range(4):
            dyn[:, 1 + k, :] = _pack(np.ascontiguousarray(r[:, k]))
        in_maps.append({
            "locst": _LOCSTAT[h],
            "locst16": _LOCSTAT16[h],
            "dyn": np.ascontiguousarray(dyn),
            "boxesT": np.ascontiguousarray(boxes[b].T),
            "boxesP": np.ascontiguousarray(np.tile(boxes[b], (2, 1))),
            "agnfull": agnfull,
            "corec": _corec(b, h),
        })
    return in_maps


_NC_CACHE = {}
LAST_RESULT = None


def _get_nc():
    if "nc" not in _NC_CACHE:
        _NC_CACHE["nc"] = build_nc(dbg=False)
    return _NC_CACHE["nc"]


def kernel(boxes, gt_classes=None, agn_hm_pred=None, reg_pred=None):
    global LAST_RESULT
    in_maps = make_in_maps(boxes, agn_hm_pred, reg_pred)
    nc = _get_nc()
    res = run_bass_kernel_spmd(nc, in_maps, core_ids=list(range(N_CORES)))
    LAST_RESULT = res
    parts = np.stack([np.asarray(r["out"], np.float64).reshape(8)
                      for r in res.results], 0).sum(0)
    rega, regc, poss, nega, npos = parts[0], parts[1], parts[2], parts[3], parts[4]
    npa = max(npos, 1.0)
    out = np.array([rega / max(regc, 1.0),
                    -0.125 * poss / npa,
                    -0.375 * nega / npa], np.float32)
    return out
